# revision 48
# baseline (speedup 1.0000x reference)
"""Trainium2 Bass kernel for a 2-layer GCN forward pass (8 NeuronCores).

    h    = relu(spmm(A, x @ W1) + b1)
    out  = softmax(spmm(A, h @ W2) + b2)   with spmm(A, h @ W2) == spmm(A, h) @ W2

Strategy (graph/data parallel over 8 cores, dst-node sharded):
  K1: node-sharded dense matmul  support = x @ W1       (per-core rows, f32 PE)
  host: all-to-all gather of source-node support rows into dst-sorted,
        degree-bucketed slot slabs (pure movement / replication)
  K2: per-core slab streaming: val-multiply (DVE+GpSimd) -> segmented
      reduce over the degree axis (DVE tensor_reduce) -> +b1, relu (ACT)
      -> hW2 = h @ W2 (PE transpose + matmul) -> hW2 shard
  host: assemble full hW2 table, gather into 16-wide slot slabs
  K3: slab streaming: val-multiply + segmented reduce -> +b2 -> softmax

Slot layout (identical across cores so one SPMD program serves all 8):
  * each core's 12500 dst nodes are sorted by in-degree (desc) and laid
    out on a [128 partitions x Q columns] grid (i-th -> p=i%128, q=i//128).
  * column q holds D_q = max-over-cores in-degree of its 128 dsts; slots
    for (p, q) are that dst's edges padded with val=0 to D_q.  Sorting
    makes D_q tight (total padding ~5%).
  * slab element (p, q, h, d) = table[src(p,q,d), h]; the device computes
    sum_d val(p,q,d) * slab(p,q,h,d) per (p, q, h) with one broadcast
    multiply and one innermost-axis tensor_reduce per chunk.
"""
import os
import sys
import time

for _p in ("/opt/trn_rl_repo", "/opt/pypackages"):
    if _p not in sys.path:
        sys.path.append(_p)

import numpy as np
from concourse import bacc, mybir, tile, bass_utils

F32 = mybir.dt.float32
BF16 = mybir.dt.bfloat16
I16 = mybir.dt.int16
AX = mybir.AxisListType.X
MUL = mybir.AluOpType.mult
ADD = mybir.AluOpType.add
EXP = mybir.ActivationFunctionType.Exp
CPY = mybir.ActivationFunctionType.Copy
RELU = mybir.ActivationFunctionType.Relu

P = 128


class Cfg:
    def __init__(self, n_nodes=100000, f_in=512, hidden=64, n_class=16,
                 n_cores=8, chunk_elems=8192, k1_cols=1024):
        self.n_nodes, self.f_in, self.hidden, self.n_class = n_nodes, f_in, hidden, n_class
        self.n_cores = n_cores
        self.chunk_elems = chunk_elems          # per-partition f32 elems per k2 chunk
        self.k1_cols = k1_cols
        assert n_nodes % n_cores == 0
        self.npc = n_nodes // n_cores
        self.Q = -(-self.npc // P)
        self.NP = self.Q * P
        assert f_in % P == 0
        self.kb = f_in // P


class Sched:
    """Static (cross-core identical) slot schedule + per-core fill arrays."""

    def __init__(self, cfg: Cfg, edge_src, edge_dst, edge_val):
        self.cfg = cfg
        ncr, npc, Q, NP = cfg.n_cores, cfg.npc, cfg.Q, cfg.NP

        core = edge_dst // npc
        dst_l = edge_dst % npc

        # per-core degree + degree-sorted dst order
        self.order = np.zeros((ncr, NP), np.int64)
        ds = np.zeros((ncr, NP), np.int64)
        for c in range(ncr):
            deg = np.bincount(dst_l[core == c], minlength=npc)
            degp = np.full(NP, -1, np.int64)
            degp[:npc] = deg
            o = np.argsort(-degp, kind="stable")
            self.order[c] = o
            ds[c] = degp[o]
        ds = np.maximum(ds, 0)

        # static per-column D = max over cores of column max (desc sort ->
        # column max is its first element); >=1 so every column is covered
        D_q = np.maximum(ds[:, ::P].max(axis=0), 1)     # [Q]
        self.D_q = D_q

        # runs of equal D
        runs = []
        q = 0
        while q < Q:
            q1 = q
            while q1 + 1 < Q and D_q[q1 + 1] == D_q[q]:
                q1 += 1
            runs.append((q, q1 + 1, int(D_q[q])))
            q = q1 + 1
        self.runs = runs

        # per-column slot offset (in D-units) for columns inside runs
        coff = np.full(Q, -1, np.int64)
        off = 0
        for (q0, q1, D) in runs:
            for qq in range(q0, q1):
                coff[qq] = off
                off += D
        self.VT = int(off)                      # per-partition slot count

        # per-core slot fill: src index + edge val per (p, q, d)
        self.srcmat = np.zeros((ncr, P, self.VT), np.int32)
        self.valmat = np.zeros((ncr, P, self.VT), np.float32)
        for c in range(ncr):
            m = core == c
            es, ev, dl = edge_src[m], edge_val[m], dst_l[m]
            so = np.argsort(dl, kind="stable")
            es, ev, dl = es[so], ev[so], dl[so]
            # within-dst rank
            first = np.r_[True, dl[1:] != dl[:-1]] if len(dl) else np.array([], bool)
            starts = np.flatnonzero(first)
            sizes = np.diff(np.r_[starts, len(dl)])
            rank = np.arange(len(dl)) - np.repeat(starts, sizes)
            # dst -> (p, q)
            pos = np.zeros(NP, np.int64)
            pos[self.order[c]] = np.arange(NP)
            pe = pos[dl] % P
            qe = pos[dl] // P
            flat = coff[qe] + rank
            self.srcmat[c, pe, flat] = es
            self.valmat[c, pe, flat] = ev

        # chunk plan (static): per run, split columns so per-partition f32
        # elems (nq*h*D) stays under cfg.chunk_elems (h = table width)
        self.coff = coff

    def chunks(self, width, chunk_elems):
        """DMA chunks packing whole run-segments.

        Returns list of (eoff, L, q0c, nqc, segs) where segs is a list of
        (qseg, nqseg, D, loc) with loc the f32 offset of the segment inside
        the chunk tile. Chunk columns [q0c, q0c+nqc) are contiguous."""
        segs_all = []
        seg_elems = max(1, chunk_elems * 3 // 8)
        for (q0, q1, D) in self.runs:
            nq_max = max(1, seg_elems // (width * D))
            q = q0
            while q < q1:
                nq = min(nq_max, q1 - q)
                segs_all.append((q, nq, D))
                q += nq
        out = []
        cur = None
        for (q, nq, D) in segs_all:
            L = nq * width * D
            if cur is not None and cur["L"] + L <= chunk_elems:
                cur["segs"].append((q, nq, D, cur["L"]))
                cur["L"] += L
                cur["nqc"] += nq
            else:
                if cur is not None:
                    out.append(cur)
                cur = dict(eoff=int(self.coff[q]) * width, L=L, q0c=q,
                           nqc=nq, segs=[(q, nq, D, 0)])
        if cur is not None:
            out.append(cur)
        return out

    def build_slab(self, core, table, width):
        """slab[p, (q, h, d)] = table[src(p, q, d), h]  (f32, [P, VT*width])"""
        sub = self.srcmat[core]                                  # [P, VT]
        g = table[sub.reshape(-1)].reshape(P, self.VT, width)    # [P, VT, w]
        out = np.empty((P, self.VT * width), table.dtype)
        for (q0, q1, D) in self.runs:
            a, b = self.coff[q0], self.coff[q0] + (q1 - q0) * D
            blk = g[:, a:b, :].reshape(P, q1 - q0, D, width)
            out[:, a * width:b * width] = (
                blk.transpose(0, 1, 3, 2).reshape(P, -1))
        return out


# ---------------------------------------------------------------- kernels
def build_k1(cfg: Cfg):
    """sup.T = (x @ W1).T via psum[64, cols] accumulation.

    f32 precision at bf16 PE rate: x and W1 are split hi/lo in bf16 and
    three of the four cross terms are accumulated (lo*lo ~ 2^-16, dropped).
    """
    H, kb, NP = cfg.hidden, cfg.kb, cfg.NP
    CC = cfg.k1_cols            # DMA chunk columns
    PC = min(512, CC)           # psum sub-chunk columns
    nc = bacc.Bacc(None, target_bir_lowering=False)
    x_d = nc.dram_tensor("xhl", [P, kb, 2, NP], BF16, kind="ExternalInput")
    w1_d = nc.dram_tensor("w1hl", [P, kb, 2, H], BF16, kind="ExternalInput")
    sup_d = nc.dram_tensor("sup", [H, NP], F32, kind="ExternalOutput")

    n_ch = -(-NP // CC)
    with tile.TileContext(nc) as tc:
        with (
            tc.tile_pool(name="const", bufs=1) as cpool,
            tc.tile_pool(name="xload", bufs=6) as xpool,
            tc.tile_pool(name="sout", bufs=1) as opool,
            tc.tile_pool(name="ps", bufs=5, space="PSUM") as pspool,
            tc.tile_pool(name="psw", bufs=1, space="PSUM") as pswarm,
        ):
            w1_t = cpool.tile([P, kb, 2, H], BF16)
            nc.sync.dma_start(w1_t[:], w1_d[:])
            osb = opool.tile([H, NP], F32)
            # ~4.5us of dummy matmuls while the first x chunk is in flight:
            # sustained PE activity flips the HAM clock gate 1.2 -> 2.4 GHz
            # before the real matmuls start (stays warm; PE runs near
            # continuously afterwards).
            ps_w = pswarm.tile([H, H], F32, tag="warm")
            for _ in range(80):
                nc.tensor.matmul(ps_w[:], w1_t[:, 0, 0, :], w1_t[:, 0, 0, :],
                                 start=True, stop=True)
            for i in range(n_ch):
                c0 = i * CC
                ncols = min(CC, NP - c0)
                xc = xpool.tile([P, kb, 2, CC], BF16, tag="xc")
                nc.sync.dma_start(xc[:, :, :, :ncols],
                                  x_d[:, :, :, c0:c0 + ncols])
                for s0 in range(0, ncols, PC):
                    sc = min(PC, ncols - s0)
                    ps = pspool.tile([H, PC], F32, tag="ps")
                    nmm = 3 * kb
                    m = 0
                    for k in range(kb):
                        for hl in (0, 1):           # whi @ {xhi, xlo}
                            nc.tensor.matmul(ps[:, :sc], w1_t[:, k, 0, :],
                                             xc[:, k, hl, s0:s0 + sc],
                                             start=(m == 0), stop=(m == nmm - 1))
                            m += 1
                    for k in range(kb):             # wlo @ xhi
                        nc.tensor.matmul(ps[:, :sc], w1_t[:, k, 1, :],
                                         xc[:, k, 0, s0:s0 + sc],
                                         start=False, stop=(m == nmm - 1))
                        m += 1
                    nc.scalar.activation(osb[:, c0 + s0:c0 + s0 + sc],
                                         ps[:, :sc], CPY)
            nc.sync.dma_start(sup_d[:], osb[:])
    nc.compile()
    return nc


def build_spmm(cfg: Cfg, sch: Sched, layer: int, q_scale: float = 1.0):
    """Slab-streaming spmm. layer=1: +b1, relu, @W2 -> hW2 shard.
    layer=2: +b2, softmax -> out shard."""
    H, C, Q = cfg.hidden, cfg.n_class, cfg.Q
    W = H if layer == 1 else C          # table width
    SLT = I16 if layer == 2 else F32   # int16 halves the k3 slab DMA
    nc = bacc.Bacc(None, target_bir_lowering=False)
    slt_d = nc.dram_tensor("slots", [P, max(sch.VT * W, 1)], SLT,
                           kind="ExternalInput")
    val_d = nc.dram_tensor("valv", [P, max(sch.VT, 1)], F32,
                           kind="ExternalInput")
    if layer == 1:
        b_d = nc.dram_tensor("b1r", [P, H], F32, kind="ExternalInput")
        id_d = nc.dram_tensor("ident", [P, P], F32, kind="ExternalInput")
        w2_d = nc.dram_tensor("w2", [P, 2, C], F32, kind="ExternalInput")
        out_d = nc.dram_tensor("hw2", [P, Q * C], F32, kind="ExternalOutput")
    else:
        b_d = nc.dram_tensor("b2r", [P, C], F32, kind="ExternalInput")
        out_d = nc.dram_tensor("oout", [P, Q * C], F32, kind="ExternalOutput")

    chunks = sch.chunks(W, cfg.chunk_elems)
    nqc_max = max(ch["nqc"] for ch in chunks)
    L_max = max(ch["L"] for ch in chunks)
    seg_max = max(nq * W * D for ch in chunks for (_, nq, D, _) in ch["segs"])

    # greedy balance of the val-multiplies between GpSimd (~1.92 ns/elem,
    # ~2.5us drain overhead per op) and DVE (~1.04 ns/elem + ~0.3us/op,
    # which also owns every reduction)
    GP_NS, DVE_NS, RED_NS, GP_OP, DVE_OP = 1.55, 1.04, 0.72, 1000.0, 300.0
    gp_busy = 0.0
    dve_busy = 15000.0 if layer == 2 else 2000.0    # epilogue handicap
    mult_on_gp = []
    for ch in chunks:
        for (qseg, nq, D, loc) in ch["segs"]:
            E = nq * W * D
            dve_busy += E * RED_NS + DVE_OP         # the reduce
            gp_c = E * GP_NS + GP_OP
            dve_c = E * DVE_NS + DVE_OP
            if gp_busy + gp_c <= dve_busy + dve_c:
                mult_on_gp.append(True)
                gp_busy += gp_c
            else:
                mult_on_gp.append(False)
                dve_busy += dve_c
    with tile.TileContext(nc) as tc:
        with (
            tc.tile_pool(name="const", bufs=1) as cpool,
            tc.tile_pool(name="sld", bufs=10) as spool,
            tc.tile_pool(name="prod", bufs=8) as ppool,
            tc.tile_pool(name="acc", bufs=3) as apool,
            tc.tile_pool(name="epi", bufs=3) as epool,
            tc.tile_pool(name="ob", bufs=1) as opool,
            tc.tile_pool(name="psA", bufs=4, space="PSUM") as psA,
            tc.tile_pool(name="psB", bufs=4, space="PSUM") as psB,
        ):
            val_t = cpool.tile([P, max(sch.VT, 1)], F32)
            nc.sync.dma_start(val_t[:], val_d[:])
            if layer == 2:
                # fold the int16 dequant scale into the edge values
                nc.vector.tensor_scalar_mul(val_t[:], val_t[:],
                                            q_scale / 32768.0)
            b_t = cpool.tile([P, H if layer == 1 else C], F32)
            nc.sync.dma_start(b_t[:], b_d[:])
            if layer == 1:
                id_t = cpool.tile([P, P], F32)
                w2_t = cpool.tile([P, 2, C], F32)
                nc.sync.dma_start(id_t[:], id_d[:])
                nc.sync.dma_start(w2_t[:], w2_d[:])
            ob = opool.tile([P, Q, C], F32)
            if layer == 2:
                lg = opool.tile([P, Q, C], F32)
            else:
                hb = opool.tile([P, Q, H], F32)

            def finish_chunk(ch, acc_c):
                """Per-chunk epilogue once all its reduces are emitted."""
                q0c, nqc = ch["q0c"], ch["nqc"]
                # +b1 into the global h tile (frees acc_c immediately; PE/ACT
                # lag can't back-pressure the reduce pipeline), relu, @W2.
                hv = hb[:, q0c:q0c + nqc, :]
                nc.vector.tensor_tensor(
                    hv, acc_c[:, :nqc, :],
                    b_t[:].unsqueeze(1).broadcast_to([P, nqc, W]), op=ADD)
                nc.scalar.activation(
                    hv.rearrange("p q w -> p (q w)"),
                    hv.rearrange("p q w -> p (q w)"), RELU)
                for jj in range(0, nqc, 2):
                    nj = min(2, nqc - jj)
                    # one transpose covers two h columns (F=128)
                    ps2 = psA.tile([P, P], F32, tag="tr")
                    nc.tensor.transpose(
                        ps2[:nj * H, :],
                        hb[:, q0c + jj:q0c + jj + nj, :].rearrange(
                            "p a b -> p (a b)"), id_t[:])
                    hT = epool.tile([P, P], F32, tag="hT")
                    nc.scalar.activation(hT[:nj * H, :], ps2[:nj * H, :], CPY)
                    ps3 = psB.tile([P, 2, C], F32, tag="mm")
                    for j in range(nj):
                        nc.tensor.matmul(ps3[:, j, :], hT[:, :],
                                         w2_t[:, j, :], start=True, stop=True)
                    nc.scalar.activation(ob[:, q0c + jj:q0c + jj + nj, :],
                                         ps3[:, :nj, :], CPY)

            # software pipeline at segment granularity: each segment gets
            # its own DMA + multiply; its reduce is emitted LAG segments
            # later so neither engine head-blocks on a lagging producer.
            LAG = 6
            from collections import deque
            segq = deque()
            grp = {}

            def drain_one():
                ci, qseg, nq, D, sv = segq.popleft()
                ch = chunks[ci]
                if layer == 1:
                    acc_c, left = grp[ci]
                    dst = acc_c[:, qseg - ch["q0c"]:qseg - ch["q0c"] + nq, :]
                else:
                    dst = lg[:, qseg:qseg + nq, :]
                nc.vector.tensor_reduce(dst, sv, axis=AX, op=ADD)
                if layer == 1:
                    grp[ci][1] -= 1
                    if grp[ci][1] == 0:
                        finish_chunk(ch, grp.pop(ci)[0])

            seg_i = 0
            for ci, ch in enumerate(chunks):
                if layer == 1:
                    acc_c = apool.tile([P, nqc_max, W], F32, tag="acc")
                    grp[ci] = [acc_c, len(ch["segs"])]
                for (qseg, nq, D, loc) in ch["segs"]:
                    L = nq * W * D
                    sl = spool.tile([P, seg_max], SLT, tag="sl")
                    e0 = ch["eoff"] + loc
                    nc.sync.dma_start(sl[:, :L], slt_d[:, e0:e0 + L])
                    # drain a lagged reduce BEFORE this segment's multiply:
                    # its producer finished LAG segs ago, so the in-order DVE
                    # head never blocks on this segment's DMA while ready
                    # reduce work exists.
                    if len(segq) > LAG:
                        drain_one()
                    v4 = sl[:, :L].rearrange(
                        "p (q h d) -> p q h d", q=nq, h=W, d=D)
                    vw = (val_t[:, e0 // W:e0 // W + nq * D]
                          .rearrange("p (q d) -> p q d", q=nq)
                          .unsqueeze(2).broadcast_to([P, nq, W, D]))
                    if layer == 2:
                        pp = ppool.tile([P, seg_max], F32, tag="pp")
                        o4 = pp[:, :L].rearrange(
                            "p (q h d) -> p q h d", q=nq, h=W, d=D)
                    else:
                        o4 = v4
                    eng = nc.gpsimd if mult_on_gp[seg_i] else nc.vector
                    seg_i += 1
                    eng.tensor_tensor(o4, v4, vw, op=MUL)
                    segq.append((ci, qseg, nq, D, o4))
            while segq:
                drain_one()

            if layer == 2:
                flat = lg[:].rearrange("p q w -> p (q w)")
                nc.vector.tensor_tensor(
                    lg[:], lg[:],
                    b_t[:].unsqueeze(1).broadcast_to([P, Q, C]), op=ADD)
                nm = epool.tile([P, Q], F32, tag="nm")
                nc.vector.reduce_max(nm[:], lg[:], axis=AX, negate=True)
                nc.vector.tensor_tensor(
                    lg[:], lg[:],
                    nm[:].unsqueeze(2).broadcast_to([P, Q, C]), op=ADD)
                nc.scalar.activation(flat, flat, EXP)
                se = epool.tile([P, Q], F32, tag="se")
                nc.vector.reduce_sum(se[:], lg[:], axis=AX)
                ri = epool.tile([P, Q], F32, tag="ri")
                nc.vector.reciprocal(ri[:], se[:])
                nc.vector.tensor_tensor(
                    ob[:], lg[:],
                    ri[:].unsqueeze(2).broadcast_to([P, Q, C]), op=MUL)
            nc.sync.dma_start(out_d[:], ob[:].rearrange("p q c -> p (q c)"))
    nc.compile()
    return nc


# ---------------------------------------------------------------- driver
LAST_PROFILE = {}


def _run(nc, in_maps, label):
    trace = os.environ.get("GCN_PROFILE") == "1"
    t0 = time.time()
    res = bass_utils.run_bass_kernel_spmd(
        nc, in_maps, core_ids=list(range(len(in_maps))), trace=trace)
    LAST_PROFILE[label] = dict(wall_s=time.time() - t0,
                               exec_time_ns=res.exec_time_ns,
                               trace=(res.instructions_and_trace or (None, None))[1])
    return res.results


def gcn_forward(cfg: Cfg, x, edge_src, edge_dst, edge_val, W1, b1, W2, b2):
    ncr, H, C, Q, npc = cfg.n_cores, cfg.hidden, cfg.n_class, cfg.Q, cfg.npc
    x = np.asarray(x, np.float32)
    W1 = np.asarray(W1, np.float32)
    b1 = np.asarray(b1, np.float32)
    W2 = np.asarray(W2, np.float32)
    b2 = np.asarray(b2, np.float32)
    edge_src = np.asarray(edge_src, np.int64)
    edge_dst = np.asarray(edge_dst, np.int64)
    edge_val = np.asarray(edge_val, np.float32)

    t0 = time.time()
    sch = Sched(cfg, edge_src, edge_dst, edge_val)
    prep_s = time.time() - t0

    import ml_dtypes
    BF = ml_dtypes.bfloat16
    ident = np.eye(P, dtype=np.float32)
    b1r = np.tile(b1, (P, 1))
    b2r = np.tile(b2, (P, 1))
    w1r = np.ascontiguousarray(W1.reshape(cfg.kb, P, H).transpose(1, 0, 2))
    w2sel = np.zeros((P, 2, C), np.float32)
    w2sel[:H, 0] = W2
    w2sel[H:2 * H, 1] = W2
    w1hi = w1r.astype(BF)
    w1lo = (w1r - w1hi.astype(np.float32)).astype(BF)
    w1hl = np.ascontiguousarray(np.stack([w1hi, w1lo], axis=2))

    # K1: sup = x @ W1 (transposed output [H, NP] per core)
    in1 = []
    for c in range(ncr):
        xs = x[c * npc:(c + 1) * npc]
        xt = np.zeros((P, cfg.kb, cfg.NP), np.float32)
        xt[:, :, :npc] = xs.T.reshape(cfg.kb, P, npc).transpose(1, 0, 2)
        xhi = xt.astype(BF)
        xlo = (xt - xhi.astype(np.float32)).astype(BF)
        in1.append(dict(xhl=np.ascontiguousarray(np.stack([xhi, xlo], axis=2)),
                        w1hl=w1hl))
    nc1 = build_k1(cfg)
    r1 = _run(nc1, in1, "k1")

    sup = np.empty((cfg.n_nodes, H), np.float32)
    for c in range(ncr):
        sup[c * npc:(c + 1) * npc] = r1[c]["sup"].T[:npc]

    # K2: slab spmm + bias + relu + @W2
    in2 = [dict(slots=sch.build_slab(c, sup, H), valv=sch.valmat[c],
                b1r=b1r, ident=ident, w2=w2sel)
           for c in range(ncr)]
    nc2 = build_spmm(cfg, sch, 1)
    r2 = _run(nc2, in2, "k2")

    hw2 = np.empty((cfg.n_nodes, C), np.float32)
    for c in range(ncr):
        flat = r2[c]["hw2"].reshape(P, Q, C).transpose(1, 0, 2).reshape(-1, C)
        o = sch.order[c]
        m = o < npc
        hw2[c * npc + o[m]] = flat[m]

    # K3: slab spmm + bias + softmax
    s2 = float(2.0 ** np.ceil(np.log2(np.abs(hw2).max() * 1.001 + 1e-30)))
    hw2_q = np.clip(np.round(hw2 * (32768.0 / s2)), -32767, 32767).astype(np.int16)
    in3 = [dict(slots=sch.build_slab(c, hw2_q, C), valv=sch.valmat[c], b2r=b2r)
           for c in range(ncr)]
    nc3 = build_spmm(cfg, sch, 2, q_scale=s2)
    r3 = _run(nc3, in3, "k3")

    out = np.empty((cfg.n_nodes, C), np.float32)
    for c in range(ncr):
        flat = r3[c]["oout"].reshape(P, Q, C).transpose(1, 0, 2).reshape(-1, C)
        o = sch.order[c]
        m = o < npc
        out[c * npc + o[m]] = flat[m]

    LAST_PROFILE["prep_s"] = prep_s
    LAST_PROFILE["sched"] = dict(VT=sch.VT, runs=len(sch.runs),
                                 n_chunks2=len(sch.chunks(H, cfg.chunk_elems)),
                                 pad=float(sch.VT * P * ncr) / max(len(edge_src), 1))
    return out


def kernel(x, edge_src, edge_dst, edge_val, W1, b1, W2, b2):
    cfg = Cfg()
    return gcn_forward(cfg, x, edge_src, edge_dst, edge_val, W1, b1, W2, b2)


# ---------------------------------------------------------------- self test
def _numpy_ref(x, es, ed, ev, W1, b1, W2, b2, n):
    def spmm(d):
        g = d[es] * ev[:, None]
        out = np.zeros((n, d.shape[1]), np.float32)
        np.add.at(out, ed, g)
        return out
    h = spmm(x @ W1) + b1
    h = np.maximum(h, 0)
    lg = spmm(h @ W2) + b2
    e = np.exp(lg - lg.max(1, keepdims=True))
    return e / e.sum(1, keepdims=True)


def _selftest():
    cfg = Cfg(n_nodes=4096, f_in=256, hidden=64, n_class=16, n_cores=8,
              chunk_elems=2048, k1_cols=256)
    rng = np.random.default_rng(1)
    n_edges = 65536
    x = rng.standard_normal((cfg.n_nodes, cfg.f_in), dtype=np.float32)
    es = rng.integers(0, cfg.n_nodes, n_edges)
    ed = rng.integers(0, cfg.n_nodes, n_edges)
    ev = rng.random(n_edges, dtype=np.float32)
    W1 = rng.standard_normal((cfg.f_in, cfg.hidden), dtype=np.float32) * 0.125
    b1 = rng.standard_normal(cfg.hidden, dtype=np.float32) * 0.01
    W2 = rng.standard_normal((cfg.hidden, cfg.n_class), dtype=np.float32) * 0.25
    b2 = rng.standard_normal(cfg.n_class, dtype=np.float32) * 0.01
    act = gcn_forward(cfg, x, es, ed, ev, W1, b1, W2, b2)
    ref = _numpy_ref(x, es, ed, ev, W1, b1, W2, b2, cfg.n_nodes)
    err = np.abs(act - ref).max()
    rel = err / np.abs(ref).max()
    print(f"selftest absmax={err:.3e} relmax={rel:.3e}")
    print("profile:", LAST_PROFILE)
    assert rel < 5e-3, "SELFTEST FAIL"
    print("SELFTEST PASS")


if __name__ == "__main__":
    _selftest()


# revision 49
# speedup vs baseline: 1.1373x; 1.1373x over previous
"""Trainium2 Bass kernel for a 2-layer GCN forward pass (8 NeuronCores).

    h    = relu(spmm(A, x @ W1) + b1)
    out  = softmax(spmm(A, h @ W2) + b2)   with spmm(A, h @ W2) == spmm(A, h) @ W2

Strategy (graph/data parallel over 8 cores, dst-node sharded):
  K1: node-sharded dense matmul  support = x @ W1       (per-core rows, f32 PE)
  host: all-to-all gather of source-node support rows into dst-sorted,
        degree-bucketed slot slabs (pure movement / replication)
  K2: per-core slab streaming: val-multiply (DVE+GpSimd) -> segmented
      reduce over the degree axis (DVE tensor_reduce) -> +b1, relu (ACT)
      -> hW2 = h @ W2 (PE transpose + matmul) -> hW2 shard
  host: assemble full hW2 table, gather into 16-wide slot slabs
  K3: slab streaming: val-multiply + segmented reduce -> +b2 -> softmax

Slot layout (identical across cores so one SPMD program serves all 8):
  * each core's 12500 dst nodes are sorted by in-degree (desc) and laid
    out on a [128 partitions x Q columns] grid (i-th -> p=i%128, q=i//128).
  * column q holds D_q = max-over-cores in-degree of its 128 dsts; slots
    for (p, q) are that dst's edges padded with val=0 to D_q.  Sorting
    makes D_q tight (total padding ~5%).
  * slab element (p, q, h, d) = table[src(p,q,d), h]; the device computes
    sum_d val(p,q,d) * slab(p,q,h,d) per (p, q, h) with one broadcast
    multiply and one innermost-axis tensor_reduce per chunk.
"""
import os
import sys
import time

for _p in ("/opt/trn_rl_repo", "/opt/pypackages"):
    if _p not in sys.path:
        sys.path.append(_p)

import numpy as np
from concourse import bacc, mybir, tile, bass_utils

F32 = mybir.dt.float32
BF16 = mybir.dt.bfloat16
I16 = mybir.dt.int16
AX = mybir.AxisListType.X
MUL = mybir.AluOpType.mult
ADD = mybir.AluOpType.add
EXP = mybir.ActivationFunctionType.Exp
CPY = mybir.ActivationFunctionType.Copy
RELU = mybir.ActivationFunctionType.Relu

P = 128


class Cfg:
    def __init__(self, n_nodes=100000, f_in=512, hidden=64, n_class=16,
                 n_cores=8, chunk_elems=8192, k1_cols=1024):
        self.n_nodes, self.f_in, self.hidden, self.n_class = n_nodes, f_in, hidden, n_class
        self.n_cores = n_cores
        self.chunk_elems = chunk_elems          # per-partition f32 elems per k2 chunk
        self.k1_cols = k1_cols
        assert n_nodes % n_cores == 0
        self.npc = n_nodes // n_cores
        self.Q = -(-self.npc // P)
        self.NP = self.Q * P
        assert f_in % P == 0
        self.kb = f_in // P


class Sched:
    """Static (cross-core identical) slot schedule + per-core fill arrays."""

    def __init__(self, cfg: Cfg, edge_src, edge_dst, edge_val):
        self.cfg = cfg
        ncr, npc, Q, NP = cfg.n_cores, cfg.npc, cfg.Q, cfg.NP

        core = edge_dst // npc
        dst_l = edge_dst % npc

        # per-core degree + degree-sorted dst order
        self.order = np.zeros((ncr, NP), np.int64)
        ds = np.zeros((ncr, NP), np.int64)
        for c in range(ncr):
            deg = np.bincount(dst_l[core == c], minlength=npc)
            degp = np.full(NP, -1, np.int64)
            degp[:npc] = deg
            o = np.argsort(-degp, kind="stable")
            self.order[c] = o
            ds[c] = degp[o]
        ds = np.maximum(ds, 0)

        # static per-column D = max over cores of column max (desc sort ->
        # column max is its first element); >=1 so every column is covered
        D_q = np.maximum(ds[:, ::P].max(axis=0), 1)     # [Q]
        self.D_q = D_q

        # runs of equal D
        runs = []
        q = 0
        while q < Q:
            q1 = q
            while q1 + 1 < Q and D_q[q1 + 1] == D_q[q]:
                q1 += 1
            runs.append((q, q1 + 1, int(D_q[q])))
            q = q1 + 1
        self.runs = runs

        # per-column slot offset (in D-units) for columns inside runs
        coff = np.full(Q, -1, np.int64)
        off = 0
        for (q0, q1, D) in runs:
            for qq in range(q0, q1):
                coff[qq] = off
                off += D
        self.VT = int(off)                      # per-partition slot count

        # per-core slot fill: src index + edge val per (p, q, d)
        self.srcmat = np.zeros((ncr, P, self.VT), np.int32)
        self.valmat = np.zeros((ncr, P, self.VT), np.float32)
        for c in range(ncr):
            m = core == c
            es, ev, dl = edge_src[m], edge_val[m], dst_l[m]
            so = np.argsort(dl, kind="stable")
            es, ev, dl = es[so], ev[so], dl[so]
            # within-dst rank
            first = np.r_[True, dl[1:] != dl[:-1]] if len(dl) else np.array([], bool)
            starts = np.flatnonzero(first)
            sizes = np.diff(np.r_[starts, len(dl)])
            rank = np.arange(len(dl)) - np.repeat(starts, sizes)
            # dst -> (p, q)
            pos = np.zeros(NP, np.int64)
            pos[self.order[c]] = np.arange(NP)
            pe = pos[dl] % P
            qe = pos[dl] // P
            flat = coff[qe] + rank
            self.srcmat[c, pe, flat] = es
            self.valmat[c, pe, flat] = ev

        # chunk plan (static): per run, split columns so per-partition f32
        # elems (nq*h*D) stays under cfg.chunk_elems (h = table width)
        self.coff = coff

    def chunks(self, width, chunk_elems):
        """DMA chunks packing whole run-segments.

        Returns list of (eoff, L, q0c, nqc, segs) where segs is a list of
        (qseg, nqseg, D, loc) with loc the f32 offset of the segment inside
        the chunk tile. Chunk columns [q0c, q0c+nqc) are contiguous."""
        segs_all = []
        seg_elems = max(1, chunk_elems * 3 // 8)
        for (q0, q1, D) in self.runs:
            nq_max = max(1, seg_elems // (width * D))
            q = q0
            while q < q1:
                nq = min(nq_max, q1 - q)
                segs_all.append((q, nq, D))
                q += nq
        out = []
        cur = None
        for (q, nq, D) in segs_all:
            L = nq * width * D
            if cur is not None and cur["L"] + L <= chunk_elems:
                cur["segs"].append((q, nq, D, cur["L"]))
                cur["L"] += L
                cur["nqc"] += nq
            else:
                if cur is not None:
                    out.append(cur)
                cur = dict(eoff=int(self.coff[q]) * width, L=L, q0c=q,
                           nqc=nq, segs=[(q, nq, D, 0)])
        if cur is not None:
            out.append(cur)
        return out

    def build_slab(self, core, table, width):
        """slab[p, (q, h, d)] = table[src(p, q, d), h]  (f32, [P, VT*width])"""
        sub = self.srcmat[core]                                  # [P, VT]
        g = table[sub.reshape(-1)].reshape(P, self.VT, width)    # [P, VT, w]
        out = np.empty((P, self.VT * width), table.dtype)
        for (q0, q1, D) in self.runs:
            a, b = self.coff[q0], self.coff[q0] + (q1 - q0) * D
            blk = g[:, a:b, :].reshape(P, q1 - q0, D, width)
            out[:, a * width:b * width] = (
                blk.transpose(0, 1, 3, 2).reshape(P, -1))
        return out


# ---------------------------------------------------------------- kernels
def build_k1(cfg: Cfg):
    """sup.T = (x @ W1).T via psum[64, cols] accumulation.

    f32 precision at bf16 PE rate: x and W1 are split hi/lo in bf16 and
    three of the four cross terms are accumulated (lo*lo ~ 2^-16, dropped).
    """
    H, kb, NP = cfg.hidden, cfg.kb, cfg.NP
    CC = cfg.k1_cols            # DMA chunk columns
    PC = min(512, CC)           # psum sub-chunk columns
    nc = bacc.Bacc(None, target_bir_lowering=False)
    x_d = nc.dram_tensor("xhl", [P, kb, 2, NP], BF16, kind="ExternalInput")
    w1_d = nc.dram_tensor("w1hl", [P, kb, 2, H], BF16, kind="ExternalInput")
    sup_d = nc.dram_tensor("sup", [H, NP], F32, kind="ExternalOutput")

    n_ch = -(-NP // CC)
    with tile.TileContext(nc) as tc:
        with (
            tc.tile_pool(name="const", bufs=1) as cpool,
            tc.tile_pool(name="xload", bufs=6) as xpool,
            tc.tile_pool(name="sout", bufs=1) as opool,
            tc.tile_pool(name="ps", bufs=5, space="PSUM") as pspool,
            tc.tile_pool(name="psw", bufs=1, space="PSUM") as pswarm,
        ):
            w1_t = cpool.tile([P, kb, 2, H], BF16)
            nc.sync.dma_start(w1_t[:], w1_d[:])
            osb = opool.tile([H, NP], F32)
            # ~4.5us of dummy matmuls while the first x chunk is in flight:
            # sustained PE activity flips the HAM clock gate 1.2 -> 2.4 GHz
            # before the real matmuls start (stays warm; PE runs near
            # continuously afterwards).
            ps_w = pswarm.tile([H, H], F32, tag="warm")
            for _ in range(80):
                nc.tensor.matmul(ps_w[:], w1_t[:, 0, 0, :], w1_t[:, 0, 0, :],
                                 start=True, stop=True)
            for i in range(n_ch):
                c0 = i * CC
                ncols = min(CC, NP - c0)
                xc = xpool.tile([P, kb, 2, CC], BF16, tag="xc")
                nc.sync.dma_start(xc[:, :, :, :ncols],
                                  x_d[:, :, :, c0:c0 + ncols])
                for s0 in range(0, ncols, PC):
                    sc = min(PC, ncols - s0)
                    ps = pspool.tile([H, PC], F32, tag="ps")
                    nmm = 3 * kb
                    m = 0
                    for k in range(kb):
                        for hl in (0, 1):           # whi @ {xhi, xlo}
                            nc.tensor.matmul(ps[:, :sc], w1_t[:, k, 0, :],
                                             xc[:, k, hl, s0:s0 + sc],
                                             start=(m == 0), stop=(m == nmm - 1))
                            m += 1
                    for k in range(kb):             # wlo @ xhi
                        nc.tensor.matmul(ps[:, :sc], w1_t[:, k, 1, :],
                                         xc[:, k, 0, s0:s0 + sc],
                                         start=False, stop=(m == nmm - 1))
                        m += 1
                    nc.scalar.activation(osb[:, c0 + s0:c0 + s0 + sc],
                                         ps[:, :sc], CPY)
            nc.sync.dma_start(sup_d[:], osb[:])
    nc.compile()
    return nc


def build_spmm(cfg: Cfg, sch: Sched, layer: int, q_scale: float = 1.0):
    """Slab-streaming spmm. layer=1: +b1, relu, @W2 -> hW2 shard.
    layer=2: +b2, softmax -> out shard."""
    H, C, Q = cfg.hidden, cfg.n_class, cfg.Q
    W = H if layer == 1 else C          # table width
    SLT = F32
    nc = bacc.Bacc(None, target_bir_lowering=False)
    slt_d = nc.dram_tensor("slots", [P, max(sch.VT * W, 1)], SLT,
                           kind="ExternalInput")
    val_d = nc.dram_tensor("valv", [P, max(sch.VT, 1)], F32,
                           kind="ExternalInput")
    if layer == 1:
        b_d = nc.dram_tensor("b1r", [P, H], F32, kind="ExternalInput")
        id_d = nc.dram_tensor("ident", [P, P], F32, kind="ExternalInput")
        w2_d = nc.dram_tensor("w2", [P, 2, C], F32, kind="ExternalInput")
        out_d = nc.dram_tensor("hw2", [P, Q * C], F32, kind="ExternalOutput")
    else:
        b_d = nc.dram_tensor("b2r", [P, C], F32, kind="ExternalInput")
        out_d = nc.dram_tensor("oout", [P, Q * C], F32, kind="ExternalOutput")

    chunks = sch.chunks(W, cfg.chunk_elems)
    nqc_max = max(ch["nqc"] for ch in chunks)
    L_max = max(ch["L"] for ch in chunks)
    seg_max = max(nq * W * D for ch in chunks for (_, nq, D, _) in ch["segs"])

    # greedy balance of the val-multiplies between GpSimd (~1.92 ns/elem,
    # ~2.5us drain overhead per op) and DVE (~1.04 ns/elem + ~0.3us/op,
    # which also owns every reduction)
    GP_NS, DVE_NS, RED_NS, GP_OP, DVE_OP = 1.55, 1.04, 0.72, 1000.0, 300.0
    gp_busy = 0.0
    dve_busy = 15000.0 if layer == 2 else 2000.0    # epilogue handicap
    mult_on_gp = []
    for ch in chunks:
        for (qseg, nq, D, loc) in ch["segs"]:
            E = nq * W * D
            dve_busy += E * RED_NS + DVE_OP         # the reduce
            gp_c = E * GP_NS + GP_OP
            dve_c = E * DVE_NS + DVE_OP
            if gp_busy + gp_c <= dve_busy + dve_c:
                mult_on_gp.append(True)
                gp_busy += gp_c
            else:
                mult_on_gp.append(False)
                dve_busy += dve_c
    with tile.TileContext(nc) as tc:
        with (
            tc.tile_pool(name="const", bufs=1) as cpool,
            tc.tile_pool(name="sld", bufs=10) as spool,
            tc.tile_pool(name="acc", bufs=3) as apool,
            tc.tile_pool(name="epi", bufs=3) as epool,
            tc.tile_pool(name="ob", bufs=1) as opool,
            tc.tile_pool(name="psA", bufs=4, space="PSUM") as psA,
            tc.tile_pool(name="psB", bufs=4, space="PSUM") as psB,
        ):
            val_t = cpool.tile([P, max(sch.VT, 1)], F32)
            nc.sync.dma_start(val_t[:], val_d[:])
            b_t = cpool.tile([P, H if layer == 1 else C], F32)
            nc.sync.dma_start(b_t[:], b_d[:])
            if layer == 1:
                id_t = cpool.tile([P, P], F32)
                w2_t = cpool.tile([P, 2, C], F32)
                nc.sync.dma_start(id_t[:], id_d[:])
                nc.sync.dma_start(w2_t[:], w2_d[:])
            ob = opool.tile([P, Q, C], F32)
            if layer == 2:
                lg = opool.tile([P, Q, C], F32)
            else:
                hb = opool.tile([P, Q, H], F32)

            def finish_chunk(ch, acc_c):
                """Per-chunk epilogue once all its reduces are emitted."""
                q0c, nqc = ch["q0c"], ch["nqc"]
                # +b1 into the global h tile (frees acc_c immediately; PE/ACT
                # lag can't back-pressure the reduce pipeline), relu, @W2.
                hv = hb[:, q0c:q0c + nqc, :]
                nc.vector.tensor_tensor(
                    hv, acc_c[:, :nqc, :],
                    b_t[:].unsqueeze(1).broadcast_to([P, nqc, W]), op=ADD)
                nc.scalar.activation(
                    hv.rearrange("p q w -> p (q w)"),
                    hv.rearrange("p q w -> p (q w)"), RELU)
                for jj in range(0, nqc, 2):
                    nj = min(2, nqc - jj)
                    # one transpose covers two h columns (F=128)
                    ps2 = psA.tile([P, P], F32, tag="tr")
                    nc.tensor.transpose(
                        ps2[:nj * H, :],
                        hb[:, q0c + jj:q0c + jj + nj, :].rearrange(
                            "p a b -> p (a b)"), id_t[:])
                    hT = epool.tile([P, P], F32, tag="hT")
                    nc.scalar.activation(hT[:nj * H, :], ps2[:nj * H, :], CPY)
                    ps3 = psB.tile([P, 2, C], F32, tag="mm")
                    for j in range(nj):
                        nc.tensor.matmul(ps3[:, j, :], hT[:, :],
                                         w2_t[:, j, :], start=True, stop=True)
                    nc.scalar.activation(ob[:, q0c + jj:q0c + jj + nj, :],
                                         ps3[:, :nj, :], CPY)

            # software pipeline at segment granularity: each segment gets
            # its own DMA + multiply; its reduce is emitted LAG segments
            # later so neither engine head-blocks on a lagging producer.
            LAG = 6
            from collections import deque
            segq = deque()
            grp = {}

            def drain_one():
                ci, qseg, nq, D, sv = segq.popleft()
                ch = chunks[ci]
                if layer == 1:
                    acc_c, left = grp[ci]
                    dst = acc_c[:, qseg - ch["q0c"]:qseg - ch["q0c"] + nq, :]
                else:
                    dst = lg[:, qseg:qseg + nq, :]
                nc.vector.tensor_reduce(dst, sv, axis=AX, op=ADD)
                if layer == 1:
                    grp[ci][1] -= 1
                    if grp[ci][1] == 0:
                        finish_chunk(ch, grp.pop(ci)[0])

            seg_i = 0
            for ci, ch in enumerate(chunks):
                if layer == 1:
                    acc_c = apool.tile([P, nqc_max, W], F32, tag="acc")
                    grp[ci] = [acc_c, len(ch["segs"])]
                for (qseg, nq, D, loc) in ch["segs"]:
                    L = nq * W * D
                    sl = spool.tile([P, seg_max], SLT, tag="sl")
                    e0 = ch["eoff"] + loc
                    nc.sync.dma_start(sl[:, :L], slt_d[:, e0:e0 + L])
                    # drain a lagged reduce BEFORE this segment's multiply:
                    # its producer finished LAG segs ago, so the in-order DVE
                    # head never blocks on this segment's DMA while ready
                    # reduce work exists.
                    if len(segq) > LAG:
                        drain_one()
                    v4 = sl[:, :L].rearrange(
                        "p (q h d) -> p q h d", q=nq, h=W, d=D)
                    vw = (val_t[:, e0 // W:e0 // W + nq * D]
                          .rearrange("p (q d) -> p q d", q=nq)
                          .unsqueeze(2).broadcast_to([P, nq, W, D]))
                    o4 = v4
                    eng = nc.gpsimd if mult_on_gp[seg_i] else nc.vector
                    seg_i += 1
                    eng.tensor_tensor(o4, v4, vw, op=MUL)
                    segq.append((ci, qseg, nq, D, o4))
            while segq:
                drain_one()

            if layer == 2:
                flat = lg[:].rearrange("p q w -> p (q w)")
                nc.vector.tensor_tensor(
                    lg[:], lg[:],
                    b_t[:].unsqueeze(1).broadcast_to([P, Q, C]), op=ADD)
                nm = epool.tile([P, Q], F32, tag="nm")
                nc.vector.reduce_max(nm[:], lg[:], axis=AX, negate=True)
                nc.vector.tensor_tensor(
                    lg[:], lg[:],
                    nm[:].unsqueeze(2).broadcast_to([P, Q, C]), op=ADD)
                nc.scalar.activation(flat, flat, EXP)
                se = epool.tile([P, Q], F32, tag="se")
                nc.vector.reduce_sum(se[:], lg[:], axis=AX)
                ri = epool.tile([P, Q], F32, tag="ri")
                nc.vector.reciprocal(ri[:], se[:])
                nc.vector.tensor_tensor(
                    ob[:], lg[:],
                    ri[:].unsqueeze(2).broadcast_to([P, Q, C]), op=MUL)
            nc.sync.dma_start(out_d[:], ob[:].rearrange("p q c -> p (q c)"))
    nc.compile()
    return nc


# ---------------------------------------------------------------- driver
LAST_PROFILE = {}


def _run(nc, in_maps, label):
    trace = os.environ.get("GCN_PROFILE") == "1"
    t0 = time.time()
    res = bass_utils.run_bass_kernel_spmd(
        nc, in_maps, core_ids=list(range(len(in_maps))), trace=trace)
    LAST_PROFILE[label] = dict(wall_s=time.time() - t0,
                               exec_time_ns=res.exec_time_ns,
                               trace=(res.instructions_and_trace or (None, None))[1])
    return res.results


def gcn_forward(cfg: Cfg, x, edge_src, edge_dst, edge_val, W1, b1, W2, b2):
    ncr, H, C, Q, npc = cfg.n_cores, cfg.hidden, cfg.n_class, cfg.Q, cfg.npc
    x = np.asarray(x, np.float32)
    W1 = np.asarray(W1, np.float32)
    b1 = np.asarray(b1, np.float32)
    W2 = np.asarray(W2, np.float32)
    b2 = np.asarray(b2, np.float32)
    edge_src = np.asarray(edge_src, np.int64)
    edge_dst = np.asarray(edge_dst, np.int64)
    edge_val = np.asarray(edge_val, np.float32)

    t0 = time.time()
    sch = Sched(cfg, edge_src, edge_dst, edge_val)
    prep_s = time.time() - t0

    import ml_dtypes
    BF = ml_dtypes.bfloat16
    ident = np.eye(P, dtype=np.float32)
    b1r = np.tile(b1, (P, 1))
    b2r = np.tile(b2, (P, 1))
    w1r = np.ascontiguousarray(W1.reshape(cfg.kb, P, H).transpose(1, 0, 2))
    w2sel = np.zeros((P, 2, C), np.float32)
    w2sel[:H, 0] = W2
    w2sel[H:2 * H, 1] = W2
    w1hi = w1r.astype(BF)
    w1lo = (w1r - w1hi.astype(np.float32)).astype(BF)
    w1hl = np.ascontiguousarray(np.stack([w1hi, w1lo], axis=2))

    # K1: sup = x @ W1 (transposed output [H, NP] per core)
    in1 = []
    for c in range(ncr):
        xs = x[c * npc:(c + 1) * npc]
        xt = np.zeros((P, cfg.kb, cfg.NP), np.float32)
        xt[:, :, :npc] = xs.T.reshape(cfg.kb, P, npc).transpose(1, 0, 2)
        xhi = xt.astype(BF)
        xlo = (xt - xhi.astype(np.float32)).astype(BF)
        in1.append(dict(xhl=np.ascontiguousarray(np.stack([xhi, xlo], axis=2)),
                        w1hl=w1hl))
    nc1 = build_k1(cfg)
    r1 = _run(nc1, in1, "k1")

    sup = np.empty((cfg.n_nodes, H), np.float32)
    for c in range(ncr):
        sup[c * npc:(c + 1) * npc] = r1[c]["sup"].T[:npc]

    # K2: slab spmm + bias + relu + @W2
    in2 = [dict(slots=sch.build_slab(c, sup, H), valv=sch.valmat[c],
                b1r=b1r, ident=ident, w2=w2sel)
           for c in range(ncr)]
    nc2 = build_spmm(cfg, sch, 1)
    r2 = _run(nc2, in2, "k2")

    hw2 = np.empty((cfg.n_nodes, C), np.float32)
    for c in range(ncr):
        flat = r2[c]["hw2"].reshape(P, Q, C).transpose(1, 0, 2).reshape(-1, C)
        o = sch.order[c]
        m = o < npc
        hw2[c * npc + o[m]] = flat[m]

    # K3: slab spmm + bias + softmax
    in3 = [dict(slots=sch.build_slab(c, hw2, C), valv=sch.valmat[c], b2r=b2r)
           for c in range(ncr)]
    nc3 = build_spmm(cfg, sch, 2)
    r3 = _run(nc3, in3, "k3")

    out = np.empty((cfg.n_nodes, C), np.float32)
    for c in range(ncr):
        flat = r3[c]["oout"].reshape(P, Q, C).transpose(1, 0, 2).reshape(-1, C)
        o = sch.order[c]
        m = o < npc
        out[c * npc + o[m]] = flat[m]

    LAST_PROFILE["prep_s"] = prep_s
    LAST_PROFILE["sched"] = dict(VT=sch.VT, runs=len(sch.runs),
                                 n_chunks2=len(sch.chunks(H, cfg.chunk_elems)),
                                 pad=float(sch.VT * P * ncr) / max(len(edge_src), 1))
    return out


def kernel(x, edge_src, edge_dst, edge_val, W1, b1, W2, b2):
    cfg = Cfg()
    return gcn_forward(cfg, x, edge_src, edge_dst, edge_val, W1, b1, W2, b2)


# ---------------------------------------------------------------- self test
def _numpy_ref(x, es, ed, ev, W1, b1, W2, b2, n):
    def spmm(d):
        g = d[es] * ev[:, None]
        out = np.zeros((n, d.shape[1]), np.float32)
        np.add.at(out, ed, g)
        return out
    h = spmm(x @ W1) + b1
    h = np.maximum(h, 0)
    lg = spmm(h @ W2) + b2
    e = np.exp(lg - lg.max(1, keepdims=True))
    return e / e.sum(1, keepdims=True)


def _selftest():
    cfg = Cfg(n_nodes=4096, f_in=256, hidden=64, n_class=16, n_cores=8,
              chunk_elems=2048, k1_cols=256)
    rng = np.random.default_rng(1)
    n_edges = 65536
    x = rng.standard_normal((cfg.n_nodes, cfg.f_in), dtype=np.float32)
    es = rng.integers(0, cfg.n_nodes, n_edges)
    ed = rng.integers(0, cfg.n_nodes, n_edges)
    ev = rng.random(n_edges, dtype=np.float32)
    W1 = rng.standard_normal((cfg.f_in, cfg.hidden), dtype=np.float32) * 0.125
    b1 = rng.standard_normal(cfg.hidden, dtype=np.float32) * 0.01
    W2 = rng.standard_normal((cfg.hidden, cfg.n_class), dtype=np.float32) * 0.25
    b2 = rng.standard_normal(cfg.n_class, dtype=np.float32) * 0.01
    act = gcn_forward(cfg, x, es, ed, ev, W1, b1, W2, b2)
    ref = _numpy_ref(x, es, ed, ev, W1, b1, W2, b2, cfg.n_nodes)
    err = np.abs(act - ref).max()
    rel = err / np.abs(ref).max()
    print(f"selftest absmax={err:.3e} relmax={rel:.3e}")
    print("profile:", LAST_PROFILE)
    assert rel < 1e-3, "SELFTEST FAIL"
    print("SELFTEST PASS")


if __name__ == "__main__":
    _selftest()


# revision 50
# speedup vs baseline: 1.1661x; 1.0253x over previous
"""Trainium2 Bass kernel for a 2-layer GCN forward pass (8 NeuronCores).

    h    = relu(spmm(A, x @ W1) + b1)
    out  = softmax(spmm(A, h @ W2) + b2)   with spmm(A, h @ W2) == spmm(A, h) @ W2

Strategy (graph/data parallel over 8 cores, dst-node sharded):
  K1: node-sharded dense matmul  support = x @ W1       (per-core rows, f32 PE)
  host: all-to-all gather of source-node support rows into dst-sorted,
        degree-bucketed slot slabs (pure movement / replication)
  K2: per-core slab streaming: val-multiply (DVE+GpSimd) -> segmented
      reduce over the degree axis (DVE tensor_reduce) -> +b1, relu (ACT)
      -> hW2 = h @ W2 (PE transpose + matmul) -> hW2 shard
  host: assemble full hW2 table, gather into 16-wide slot slabs
  K3: slab streaming: val-multiply + segmented reduce -> +b2 -> softmax

Slot layout (identical across cores so one SPMD program serves all 8):
  * each core's 12500 dst nodes are sorted by in-degree (desc) and laid
    out on a [128 partitions x Q columns] grid (i-th -> p=i%128, q=i//128).
  * column q holds D_q = max-over-cores in-degree of its 128 dsts; slots
    for (p, q) are that dst's edges padded with val=0 to D_q.  Sorting
    makes D_q tight (total padding ~5%).
  * slab element (p, q, h, d) = table[src(p,q,d), h]; the device computes
    sum_d val(p,q,d) * slab(p,q,h,d) per (p, q, h) with one broadcast
    multiply and one innermost-axis tensor_reduce per chunk.
"""
import os
import sys
import time

for _p in ("/opt/trn_rl_repo", "/opt/pypackages"):
    if _p not in sys.path:
        sys.path.append(_p)

import numpy as np
from concourse import bacc, mybir, tile, bass_utils

F32 = mybir.dt.float32
BF16 = mybir.dt.bfloat16
I16 = mybir.dt.int16
AX = mybir.AxisListType.X
MUL = mybir.AluOpType.mult
ADD = mybir.AluOpType.add
EXP = mybir.ActivationFunctionType.Exp
CPY = mybir.ActivationFunctionType.Copy
RELU = mybir.ActivationFunctionType.Relu

P = 128


class Cfg:
    def __init__(self, n_nodes=100000, f_in=512, hidden=64, n_class=16,
                 n_cores=8, chunk_elems=8192, k1_cols=1024):
        self.n_nodes, self.f_in, self.hidden, self.n_class = n_nodes, f_in, hidden, n_class
        self.n_cores = n_cores
        self.chunk_elems = chunk_elems          # per-partition f32 elems per k2 chunk
        self.k1_cols = k1_cols
        assert n_nodes % n_cores == 0
        self.npc = n_nodes // n_cores
        self.Q = -(-self.npc // P)
        self.NP = self.Q * P
        assert f_in % P == 0
        self.kb = f_in // P


class Sched:
    """Static (cross-core identical) slot schedule + per-core fill arrays."""

    def __init__(self, cfg: Cfg, edge_src, edge_dst, edge_val):
        self.cfg = cfg
        ncr, npc, Q, NP = cfg.n_cores, cfg.npc, cfg.Q, cfg.NP

        core = edge_dst // npc
        dst_l = edge_dst % npc

        # per-core degree + degree-sorted dst order
        self.order = np.zeros((ncr, NP), np.int64)
        ds = np.zeros((ncr, NP), np.int64)
        for c in range(ncr):
            deg = np.bincount(dst_l[core == c], minlength=npc)
            degp = np.full(NP, -1, np.int64)
            degp[:npc] = deg
            o = np.argsort(-degp, kind="stable")
            self.order[c] = o
            ds[c] = degp[o]
        ds = np.maximum(ds, 0)

        # static per-column D = max over cores of column max (desc sort ->
        # column max is its first element); >=1 so every column is covered
        D_q = np.maximum(ds[:, ::P].max(axis=0), 1)     # [Q]
        self.D_q = D_q

        # runs of equal D
        runs = []
        q = 0
        while q < Q:
            q1 = q
            while q1 + 1 < Q and D_q[q1 + 1] == D_q[q]:
                q1 += 1
            runs.append((q, q1 + 1, int(D_q[q])))
            q = q1 + 1
        self.runs = runs

        # per-column slot offset (in D-units) for columns inside runs
        coff = np.full(Q, -1, np.int64)
        off = 0
        for (q0, q1, D) in runs:
            for qq in range(q0, q1):
                coff[qq] = off
                off += D
        self.VT = int(off)                      # per-partition slot count

        # per-core slot fill: src index + edge val per (p, q, d)
        self.srcmat = np.zeros((ncr, P, self.VT), np.int32)
        self.valmat = np.zeros((ncr, P, self.VT), np.float32)
        for c in range(ncr):
            m = core == c
            es, ev, dl = edge_src[m], edge_val[m], dst_l[m]
            so = np.argsort(dl, kind="stable")
            es, ev, dl = es[so], ev[so], dl[so]
            # within-dst rank
            first = np.r_[True, dl[1:] != dl[:-1]] if len(dl) else np.array([], bool)
            starts = np.flatnonzero(first)
            sizes = np.diff(np.r_[starts, len(dl)])
            rank = np.arange(len(dl)) - np.repeat(starts, sizes)
            # dst -> (p, q)
            pos = np.zeros(NP, np.int64)
            pos[self.order[c]] = np.arange(NP)
            pe = pos[dl] % P
            qe = pos[dl] // P
            flat = coff[qe] + rank
            self.srcmat[c, pe, flat] = es
            self.valmat[c, pe, flat] = ev

        # chunk plan (static): per run, split columns so per-partition f32
        # elems (nq*h*D) stays under cfg.chunk_elems (h = table width)
        self.coff = coff

    def chunks(self, width, chunk_elems):
        """DMA chunks packing whole run-segments.

        Returns list of (eoff, L, q0c, nqc, segs) where segs is a list of
        (qseg, nqseg, D, loc) with loc the f32 offset of the segment inside
        the chunk tile. Chunk columns [q0c, q0c+nqc) are contiguous."""
        segs_all = []
        seg_elems = max(1, chunk_elems * 3 // 8)
        for (q0, q1, D) in self.runs:
            nq_max = max(1, seg_elems // (width * D))
            q = q0
            while q < q1:
                nq = min(nq_max, q1 - q)
                segs_all.append((q, nq, D))
                q += nq
        out = []
        cur = None
        for (q, nq, D) in segs_all:
            L = nq * width * D
            if cur is not None and cur["L"] + L <= chunk_elems:
                cur["segs"].append((q, nq, D, cur["L"]))
                cur["L"] += L
                cur["nqc"] += nq
            else:
                if cur is not None:
                    out.append(cur)
                cur = dict(eoff=int(self.coff[q]) * width, L=L, q0c=q,
                           nqc=nq, segs=[(q, nq, D, 0)])
        if cur is not None:
            out.append(cur)
        return out

    def build_slab(self, core, table, width):
        """slab[p, (q, h, d)] = table[src(p, q, d), h]  (f32, [P, VT*width])"""
        sub = self.srcmat[core]                                  # [P, VT]
        g = table[sub.reshape(-1)].reshape(P, self.VT, width)    # [P, VT, w]
        out = np.empty((P, self.VT * width), table.dtype)
        for (q0, q1, D) in self.runs:
            a, b = self.coff[q0], self.coff[q0] + (q1 - q0) * D
            blk = g[:, a:b, :].reshape(P, q1 - q0, D, width)
            out[:, a * width:b * width] = (
                blk.transpose(0, 1, 3, 2).reshape(P, -1))
        return out


# ---------------------------------------------------------------- kernels
def build_k1(cfg: Cfg):
    """sup.T = (x @ W1).T via psum[64, cols] accumulation.

    f32 precision at bf16 PE rate: x and W1 are split hi/lo in bf16 and
    three of the four cross terms are accumulated (lo*lo ~ 2^-16, dropped).
    """
    H, kb, NP = cfg.hidden, cfg.kb, cfg.NP
    CC = cfg.k1_cols            # DMA chunk columns
    PC = min(512, CC)           # psum sub-chunk columns
    nc = bacc.Bacc(None, target_bir_lowering=False)
    x_d = nc.dram_tensor("xhl", [P, kb, 2, NP], BF16, kind="ExternalInput")
    w1_d = nc.dram_tensor("w1hl", [P, kb, 2, H], BF16, kind="ExternalInput")
    sup_d = nc.dram_tensor("sup", [H, NP], F32, kind="ExternalOutput")

    n_ch = -(-NP // CC)
    with tile.TileContext(nc) as tc:
        with (
            tc.tile_pool(name="const", bufs=1) as cpool,
            tc.tile_pool(name="xload", bufs=6) as xpool,
            tc.tile_pool(name="sout", bufs=1) as opool,
            tc.tile_pool(name="ps", bufs=5, space="PSUM") as pspool,
            tc.tile_pool(name="psw", bufs=1, space="PSUM") as pswarm,
        ):
            w1_t = cpool.tile([P, kb, 2, H], BF16)
            nc.sync.dma_start(w1_t[:], w1_d[:])
            osb = opool.tile([H, NP], F32)
            # ~4.5us of dummy matmuls while the first x chunk is in flight:
            # sustained PE activity flips the HAM clock gate 1.2 -> 2.4 GHz
            # before the real matmuls start (stays warm; PE runs near
            # continuously afterwards).
            ps_w = pswarm.tile([H, H], F32, tag="warm")
            for _ in range(80):
                nc.tensor.matmul(ps_w[:], w1_t[:, 0, 0, :], w1_t[:, 0, 0, :],
                                 start=True, stop=True)
            for i in range(n_ch):
                c0 = i * CC
                ncols = min(CC, NP - c0)
                xc = xpool.tile([P, kb, 2, CC], BF16, tag="xc")
                nc.sync.dma_start(xc[:, :, :, :ncols],
                                  x_d[:, :, :, c0:c0 + ncols])
                for s0 in range(0, ncols, PC):
                    sc = min(PC, ncols - s0)
                    ps = pspool.tile([H, PC], F32, tag="ps")
                    nmm = 3 * kb
                    m = 0
                    for k in range(kb):
                        for hl in (0, 1):           # whi @ {xhi, xlo}
                            nc.tensor.matmul(ps[:, :sc], w1_t[:, k, 0, :],
                                             xc[:, k, hl, s0:s0 + sc],
                                             start=(m == 0), stop=(m == nmm - 1))
                            m += 1
                    for k in range(kb):             # wlo @ xhi
                        nc.tensor.matmul(ps[:, :sc], w1_t[:, k, 1, :],
                                         xc[:, k, 0, s0:s0 + sc],
                                         start=False, stop=(m == nmm - 1))
                        m += 1
                    nc.scalar.activation(osb[:, c0 + s0:c0 + s0 + sc],
                                         ps[:, :sc], CPY)
            nc.sync.dma_start(sup_d[:], osb[:])
    nc.compile()
    return nc


def build_spmm(cfg: Cfg, sch: Sched, layer: int, q_scale: float = 1.0):
    """Slab-streaming spmm. layer=1: +b1, relu, @W2 -> hW2 shard.
    layer=2: +b2, softmax -> out shard."""
    H, C, Q = cfg.hidden, cfg.n_class, cfg.Q
    W = H if layer == 1 else C          # table width
    SLT = F32
    nc = bacc.Bacc(None, target_bir_lowering=False)
    slt_d = nc.dram_tensor("slots", [P, max(sch.VT * W, 1)], SLT,
                           kind="ExternalInput")
    val_d = nc.dram_tensor("valv", [P, max(sch.VT, 1)], F32,
                           kind="ExternalInput")
    if layer == 1:
        b_d = nc.dram_tensor("b1r", [P, H], F32, kind="ExternalInput")
        id_d = nc.dram_tensor("ident", [P, P], F32, kind="ExternalInput")
        w2_d = nc.dram_tensor("w2", [P, 2, C], F32, kind="ExternalInput")
        out_d = nc.dram_tensor("hw2", [P, Q * C], F32, kind="ExternalOutput")
    else:
        b_d = nc.dram_tensor("b2r", [P, C], F32, kind="ExternalInput")
        out_d = nc.dram_tensor("oout", [P, Q * C], F32, kind="ExternalOutput")

    chunks = sch.chunks(W, cfg.chunk_elems)
    nqc_max = max(ch["nqc"] for ch in chunks)
    L_max = max(ch["L"] for ch in chunks)
    seg_max = max(nq * W * D for ch in chunks for (_, nq, D, _) in ch["segs"])

    # greedy balance of the val-multiplies between GpSimd (~1.92 ns/elem,
    # ~2.5us drain overhead per op) and DVE (~1.04 ns/elem + ~0.3us/op,
    # which also owns every reduction)
    GP_NS, DVE_NS, RED_NS, GP_OP, DVE_OP = 1.55, 1.04, 0.72, 1000.0, 300.0
    gp_busy = 0.0
    dve_busy = 15000.0 if layer == 2 else 2000.0    # epilogue handicap
    mult_on_gp = []
    for ch in chunks:
        for (qseg, nq, D, loc) in ch["segs"]:
            E = nq * W * D
            dve_busy += E * RED_NS + DVE_OP         # the reduce
            gp_c = E * GP_NS + GP_OP
            dve_c = E * DVE_NS + DVE_OP
            if gp_busy + gp_c <= dve_busy + dve_c:
                mult_on_gp.append(True)
                gp_busy += gp_c
            else:
                mult_on_gp.append(False)
                dve_busy += dve_c
    with tile.TileContext(nc) as tc:
        with (
            tc.tile_pool(name="const", bufs=1) as cpool,
            tc.tile_pool(name="sld", bufs=10) as spool,
            tc.tile_pool(name="acc", bufs=3) as apool,
            tc.tile_pool(name="epi", bufs=3) as epool,
            tc.tile_pool(name="ob", bufs=1) as opool,
            tc.tile_pool(name="psA", bufs=4, space="PSUM") as psA,
            tc.tile_pool(name="psB", bufs=4, space="PSUM") as psB,
        ):
            val_t = cpool.tile([P, max(sch.VT, 1)], F32)
            nc.sync.dma_start(val_t[:], val_d[:])
            b_t = cpool.tile([P, H if layer == 1 else C], F32)
            nc.sync.dma_start(b_t[:], b_d[:])
            if layer == 1:
                id_t = cpool.tile([P, P], F32)
                w2_t = cpool.tile([P, 2, C], F32)
                nc.sync.dma_start(id_t[:], id_d[:])
                nc.sync.dma_start(w2_t[:], w2_d[:])
            ob = opool.tile([P, Q, C], F32)
            if layer == 2:
                lg = opool.tile([P, Q, C], F32)
            else:
                hb = opool.tile([P, Q, H], F32)

            def finish_chunk(ch, acc_c):
                """Per-chunk epilogue once all its reduces are emitted."""
                q0c, nqc = ch["q0c"], ch["nqc"]
                # +b1 into the global h tile (frees acc_c immediately; PE/ACT
                # lag can't back-pressure the reduce pipeline), relu, @W2.
                hv = hb[:, q0c:q0c + nqc, :]
                nc.vector.tensor_tensor(
                    hv, acc_c[:, :nqc, :],
                    b_t[:].unsqueeze(1).broadcast_to([P, nqc, W]), op=ADD)
                nc.scalar.activation(
                    hv.rearrange("p q w -> p (q w)"),
                    hv.rearrange("p q w -> p (q w)"), RELU)
                for jj in range(0, nqc, 2):
                    nj = min(2, nqc - jj)
                    # one transpose covers two h columns (F=128)
                    ps2 = psA.tile([P, P], F32, tag="tr")
                    nc.tensor.transpose(
                        ps2[:nj * H, :],
                        hb[:, q0c + jj:q0c + jj + nj, :].rearrange(
                            "p a b -> p (a b)"), id_t[:])
                    hT = epool.tile([P, P], F32, tag="hT")
                    nc.scalar.activation(hT[:nj * H, :], ps2[:nj * H, :], CPY)
                    ps3 = psB.tile([P, 2, C], F32, tag="mm")
                    for j in range(nj):
                        nc.tensor.matmul(ps3[:, j, :], hT[:, :],
                                         w2_t[:, j, :], start=True, stop=True)
                    nc.scalar.activation(ob[:, q0c + jj:q0c + jj + nj, :],
                                         ps3[:, :nj, :], CPY)

            # software pipeline at segment granularity: each segment gets
            # its own DMA + multiply; its reduce is emitted LAG segments
            # later so neither engine head-blocks on a lagging producer.
            LAG = 6 if layer == 1 else 3
            from collections import deque
            segq = deque()
            grp = {}

            def drain_one():
                ci, qseg, nq, D, sv = segq.popleft()
                ch = chunks[ci]
                if layer == 1:
                    acc_c, left = grp[ci]
                    dst = acc_c[:, qseg - ch["q0c"]:qseg - ch["q0c"] + nq, :]
                else:
                    dst = lg[:, qseg:qseg + nq, :]
                nc.vector.tensor_reduce(dst, sv, axis=AX, op=ADD)
                if layer == 1:
                    grp[ci][1] -= 1
                    if grp[ci][1] == 0:
                        finish_chunk(ch, grp.pop(ci)[0])

            seg_i = 0
            for ci, ch in enumerate(chunks):
                if layer == 1:
                    acc_c = apool.tile([P, nqc_max, W], F32, tag="acc")
                    grp[ci] = [acc_c, len(ch["segs"])]
                for (qseg, nq, D, loc) in ch["segs"]:
                    L = nq * W * D
                    sl = spool.tile([P, seg_max], SLT, tag="sl")
                    e0 = ch["eoff"] + loc
                    nc.sync.dma_start(sl[:, :L], slt_d[:, e0:e0 + L])
                    # drain a lagged reduce BEFORE this segment's multiply:
                    # its producer finished LAG segs ago, so the in-order DVE
                    # head never blocks on this segment's DMA while ready
                    # reduce work exists.
                    if len(segq) > LAG:
                        drain_one()
                    v4 = sl[:, :L].rearrange(
                        "p (q h d) -> p q h d", q=nq, h=W, d=D)
                    vw = (val_t[:, e0 // W:e0 // W + nq * D]
                          .rearrange("p (q d) -> p q d", q=nq)
                          .unsqueeze(2).broadcast_to([P, nq, W, D]))
                    o4 = v4
                    eng = nc.gpsimd if mult_on_gp[seg_i] else nc.vector
                    seg_i += 1
                    eng.tensor_tensor(o4, v4, vw, op=MUL)
                    segq.append((ci, qseg, nq, D, o4))
            while segq:
                drain_one()

            if layer == 2:
                flat = lg[:].rearrange("p q w -> p (q w)")
                nc.vector.tensor_tensor(
                    lg[:], lg[:],
                    b_t[:].unsqueeze(1).broadcast_to([P, Q, C]), op=ADD)
                nm = epool.tile([P, Q], F32, tag="nm")
                nc.vector.reduce_max(nm[:], lg[:], axis=AX, negate=True)
                nc.vector.tensor_tensor(
                    lg[:], lg[:],
                    nm[:].unsqueeze(2).broadcast_to([P, Q, C]), op=ADD)
                nc.scalar.activation(flat, flat, EXP)
                se = epool.tile([P, Q], F32, tag="se")
                nc.vector.reduce_sum(se[:], lg[:], axis=AX)
                ri = epool.tile([P, Q], F32, tag="ri")
                nc.vector.reciprocal(ri[:], se[:])
                nc.vector.tensor_tensor(
                    ob[:], lg[:],
                    ri[:].unsqueeze(2).broadcast_to([P, Q, C]), op=MUL)
            nc.sync.dma_start(out_d[:], ob[:].rearrange("p q c -> p (q c)"))
    nc.compile()
    return nc


# ---------------------------------------------------------------- driver
LAST_PROFILE = {}


def _run(nc, in_maps, label):
    trace = os.environ.get("GCN_PROFILE") == "1"
    t0 = time.time()
    res = bass_utils.run_bass_kernel_spmd(
        nc, in_maps, core_ids=list(range(len(in_maps))), trace=trace)
    LAST_PROFILE[label] = dict(wall_s=time.time() - t0,
                               exec_time_ns=res.exec_time_ns,
                               trace=(res.instructions_and_trace or (None, None))[1])
    return res.results


def gcn_forward(cfg: Cfg, x, edge_src, edge_dst, edge_val, W1, b1, W2, b2):
    ncr, H, C, Q, npc = cfg.n_cores, cfg.hidden, cfg.n_class, cfg.Q, cfg.npc
    x = np.asarray(x, np.float32)
    W1 = np.asarray(W1, np.float32)
    b1 = np.asarray(b1, np.float32)
    W2 = np.asarray(W2, np.float32)
    b2 = np.asarray(b2, np.float32)
    edge_src = np.asarray(edge_src, np.int64)
    edge_dst = np.asarray(edge_dst, np.int64)
    edge_val = np.asarray(edge_val, np.float32)

    t0 = time.time()
    sch = Sched(cfg, edge_src, edge_dst, edge_val)
    prep_s = time.time() - t0

    import ml_dtypes
    BF = ml_dtypes.bfloat16
    ident = np.eye(P, dtype=np.float32)
    b1r = np.tile(b1, (P, 1))
    b2r = np.tile(b2, (P, 1))
    w1r = np.ascontiguousarray(W1.reshape(cfg.kb, P, H).transpose(1, 0, 2))
    w2sel = np.zeros((P, 2, C), np.float32)
    w2sel[:H, 0] = W2
    w2sel[H:2 * H, 1] = W2
    w1hi = w1r.astype(BF)
    w1lo = (w1r - w1hi.astype(np.float32)).astype(BF)
    w1hl = np.ascontiguousarray(np.stack([w1hi, w1lo], axis=2))

    # K1: sup = x @ W1 (transposed output [H, NP] per core)
    in1 = []
    for c in range(ncr):
        xs = x[c * npc:(c + 1) * npc]
        xt = np.zeros((P, cfg.kb, cfg.NP), np.float32)
        xt[:, :, :npc] = xs.T.reshape(cfg.kb, P, npc).transpose(1, 0, 2)
        xhi = xt.astype(BF)
        xlo = (xt - xhi.astype(np.float32)).astype(BF)
        in1.append(dict(xhl=np.ascontiguousarray(np.stack([xhi, xlo], axis=2)),
                        w1hl=w1hl))
    nc1 = build_k1(cfg)
    r1 = _run(nc1, in1, "k1")

    sup = np.empty((cfg.n_nodes, H), np.float32)
    for c in range(ncr):
        sup[c * npc:(c + 1) * npc] = r1[c]["sup"].T[:npc]

    # K2: slab spmm + bias + relu + @W2
    in2 = [dict(slots=sch.build_slab(c, sup, H), valv=sch.valmat[c],
                b1r=b1r, ident=ident, w2=w2sel)
           for c in range(ncr)]
    nc2 = build_spmm(cfg, sch, 1)
    r2 = _run(nc2, in2, "k2")

    hw2 = np.empty((cfg.n_nodes, C), np.float32)
    for c in range(ncr):
        flat = r2[c]["hw2"].reshape(P, Q, C).transpose(1, 0, 2).reshape(-1, C)
        o = sch.order[c]
        m = o < npc
        hw2[c * npc + o[m]] = flat[m]

    # K3: slab spmm + bias + softmax
    in3 = [dict(slots=sch.build_slab(c, hw2, C), valv=sch.valmat[c], b2r=b2r)
           for c in range(ncr)]
    nc3 = build_spmm(cfg, sch, 2)
    r3 = _run(nc3, in3, "k3")

    out = np.empty((cfg.n_nodes, C), np.float32)
    for c in range(ncr):
        flat = r3[c]["oout"].reshape(P, Q, C).transpose(1, 0, 2).reshape(-1, C)
        o = sch.order[c]
        m = o < npc
        out[c * npc + o[m]] = flat[m]

    LAST_PROFILE["prep_s"] = prep_s
    LAST_PROFILE["sched"] = dict(VT=sch.VT, runs=len(sch.runs),
                                 n_chunks2=len(sch.chunks(H, cfg.chunk_elems)),
                                 pad=float(sch.VT * P * ncr) / max(len(edge_src), 1))
    return out


def kernel(x, edge_src, edge_dst, edge_val, W1, b1, W2, b2):
    cfg = Cfg()
    return gcn_forward(cfg, x, edge_src, edge_dst, edge_val, W1, b1, W2, b2)


# ---------------------------------------------------------------- self test
def _numpy_ref(x, es, ed, ev, W1, b1, W2, b2, n):
    def spmm(d):
        g = d[es] * ev[:, None]
        out = np.zeros((n, d.shape[1]), np.float32)
        np.add.at(out, ed, g)
        return out
    h = spmm(x @ W1) + b1
    h = np.maximum(h, 0)
    lg = spmm(h @ W2) + b2
    e = np.exp(lg - lg.max(1, keepdims=True))
    return e / e.sum(1, keepdims=True)


def _selftest():
    cfg = Cfg(n_nodes=4096, f_in=256, hidden=64, n_class=16, n_cores=8,
              chunk_elems=2048, k1_cols=256)
    rng = np.random.default_rng(1)
    n_edges = 65536
    x = rng.standard_normal((cfg.n_nodes, cfg.f_in), dtype=np.float32)
    es = rng.integers(0, cfg.n_nodes, n_edges)
    ed = rng.integers(0, cfg.n_nodes, n_edges)
    ev = rng.random(n_edges, dtype=np.float32)
    W1 = rng.standard_normal((cfg.f_in, cfg.hidden), dtype=np.float32) * 0.125
    b1 = rng.standard_normal(cfg.hidden, dtype=np.float32) * 0.01
    W2 = rng.standard_normal((cfg.hidden, cfg.n_class), dtype=np.float32) * 0.25
    b2 = rng.standard_normal(cfg.n_class, dtype=np.float32) * 0.01
    act = gcn_forward(cfg, x, es, ed, ev, W1, b1, W2, b2)
    ref = _numpy_ref(x, es, ed, ev, W1, b1, W2, b2, cfg.n_nodes)
    err = np.abs(act - ref).max()
    rel = err / np.abs(ref).max()
    print(f"selftest absmax={err:.3e} relmax={rel:.3e}")
    print("profile:", LAST_PROFILE)
    assert rel < 1e-3, "SELFTEST FAIL"
    print("SELFTEST PASS")


if __name__ == "__main__":
    _selftest()


# revision 52
# speedup vs baseline: 1.1698x; 1.0032x over previous
"""Trainium2 Bass kernel for a 2-layer GCN forward pass (8 NeuronCores).

    h    = relu(spmm(A, x @ W1) + b1)
    out  = softmax(spmm(A, h @ W2) + b2)   with spmm(A, h @ W2) == spmm(A, h) @ W2

Strategy (graph/data parallel over 8 cores, dst-node sharded):
  K1: node-sharded dense matmul  support = x @ W1       (per-core rows, f32 PE)
  host: all-to-all gather of source-node support rows into dst-sorted,
        degree-bucketed slot slabs (pure movement / replication)
  K2: per-core slab streaming: val-multiply (DVE+GpSimd) -> segmented
      reduce over the degree axis (DVE tensor_reduce) -> +b1, relu (ACT)
      -> hW2 = h @ W2 (PE transpose + matmul) -> hW2 shard
  host: assemble full hW2 table, gather into 16-wide slot slabs
  K3: slab streaming: val-multiply + segmented reduce -> +b2 -> softmax

Slot layout (identical across cores so one SPMD program serves all 8):
  * each core's 12500 dst nodes are sorted by in-degree (desc) and laid
    out on a [128 partitions x Q columns] grid (i-th -> p=i%128, q=i//128).
  * column q holds D_q = max-over-cores in-degree of its 128 dsts; slots
    for (p, q) are that dst's edges padded with val=0 to D_q.  Sorting
    makes D_q tight (total padding ~5%).
  * slab element (p, q, h, d) = table[src(p,q,d), h]; the device computes
    sum_d val(p,q,d) * slab(p,q,h,d) per (p, q, h) with one broadcast
    multiply and one innermost-axis tensor_reduce per chunk.
"""
import os
import sys
import time

for _p in ("/opt/trn_rl_repo", "/opt/pypackages"):
    if _p not in sys.path:
        sys.path.append(_p)

import numpy as np
from concourse import bacc, mybir, tile, bass_utils

F32 = mybir.dt.float32
BF16 = mybir.dt.bfloat16
I16 = mybir.dt.int16
AX = mybir.AxisListType.X
MUL = mybir.AluOpType.mult
ADD = mybir.AluOpType.add
EXP = mybir.ActivationFunctionType.Exp
CPY = mybir.ActivationFunctionType.Copy
RELU = mybir.ActivationFunctionType.Relu

P = 128


class Cfg:
    def __init__(self, n_nodes=100000, f_in=512, hidden=64, n_class=16,
                 n_cores=8, chunk_elems=8192, k1_cols=1024):
        self.n_nodes, self.f_in, self.hidden, self.n_class = n_nodes, f_in, hidden, n_class
        self.n_cores = n_cores
        self.chunk_elems = chunk_elems          # per-partition f32 elems per k2 chunk
        self.k1_cols = k1_cols
        assert n_nodes % n_cores == 0
        self.npc = n_nodes // n_cores
        self.Q = -(-self.npc // P)
        self.NP = self.Q * P
        assert f_in % P == 0
        self.kb = f_in // P


class Sched:
    """Static (cross-core identical) slot schedule + per-core fill arrays."""

    def __init__(self, cfg: Cfg, edge_src, edge_dst, edge_val):
        self.cfg = cfg
        ncr, npc, Q, NP = cfg.n_cores, cfg.npc, cfg.Q, cfg.NP

        core = edge_dst // npc
        dst_l = edge_dst % npc

        # per-core degree + degree-sorted dst order
        self.order = np.zeros((ncr, NP), np.int64)
        ds = np.zeros((ncr, NP), np.int64)
        for c in range(ncr):
            deg = np.bincount(dst_l[core == c], minlength=npc)
            degp = np.full(NP, -1, np.int64)
            degp[:npc] = deg
            o = np.argsort(-degp, kind="stable")
            self.order[c] = o
            ds[c] = degp[o]
        ds = np.maximum(ds, 0)

        # static per-column D = max over cores of column max (desc sort ->
        # column max is its first element); >=1 so every column is covered
        D_q = np.maximum(ds[:, ::P].max(axis=0), 1)     # [Q]
        self.D_q = D_q

        # runs of equal D
        runs = []
        q = 0
        while q < Q:
            q1 = q
            while q1 + 1 < Q and D_q[q1 + 1] == D_q[q]:
                q1 += 1
            runs.append((q, q1 + 1, int(D_q[q])))
            q = q1 + 1
        self.runs = runs

        # per-column slot offset (in D-units) for columns inside runs
        coff = np.full(Q, -1, np.int64)
        off = 0
        for (q0, q1, D) in runs:
            for qq in range(q0, q1):
                coff[qq] = off
                off += D
        self.VT = int(off)                      # per-partition slot count

        # per-core slot fill: src index + edge val per (p, q, d)
        self.srcmat = np.zeros((ncr, P, self.VT), np.int32)
        self.valmat = np.zeros((ncr, P, self.VT), np.float32)
        for c in range(ncr):
            m = core == c
            es, ev, dl = edge_src[m], edge_val[m], dst_l[m]
            so = np.argsort(dl, kind="stable")
            es, ev, dl = es[so], ev[so], dl[so]
            # within-dst rank
            first = np.r_[True, dl[1:] != dl[:-1]] if len(dl) else np.array([], bool)
            starts = np.flatnonzero(first)
            sizes = np.diff(np.r_[starts, len(dl)])
            rank = np.arange(len(dl)) - np.repeat(starts, sizes)
            # dst -> (p, q)
            pos = np.zeros(NP, np.int64)
            pos[self.order[c]] = np.arange(NP)
            pe = pos[dl] % P
            qe = pos[dl] // P
            flat = coff[qe] + rank
            self.srcmat[c, pe, flat] = es
            self.valmat[c, pe, flat] = ev

        # chunk plan (static): per run, split columns so per-partition f32
        # elems (nq*h*D) stays under cfg.chunk_elems (h = table width)
        self.coff = coff

    def chunks(self, width, chunk_elems):
        """DMA chunks packing whole run-segments.

        Returns list of (eoff, L, q0c, nqc, segs) where segs is a list of
        (qseg, nqseg, D, loc) with loc the f32 offset of the segment inside
        the chunk tile. Chunk columns [q0c, q0c+nqc) are contiguous."""
        segs_all = []
        seg_elems = max(1, chunk_elems * 3 // 8)
        for (q0, q1, D) in self.runs:
            nq_max = max(1, seg_elems // (width * D))
            q = q0
            while q < q1:
                nq = min(nq_max, q1 - q)
                segs_all.append((q, nq, D))
                q += nq
        out = []
        cur = None
        for (q, nq, D) in segs_all:
            L = nq * width * D
            if cur is not None and cur["L"] + L <= chunk_elems:
                cur["segs"].append((q, nq, D, cur["L"]))
                cur["L"] += L
                cur["nqc"] += nq
            else:
                if cur is not None:
                    out.append(cur)
                cur = dict(eoff=int(self.coff[q]) * width, L=L, q0c=q,
                           nqc=nq, segs=[(q, nq, D, 0)])
        if cur is not None:
            out.append(cur)
        return out

    def build_slab(self, core, table, width):
        """slab[p, (q, h, d)] = table[src(p, q, d), h]  (f32, [P, VT*width])"""
        sub = self.srcmat[core]                                  # [P, VT]
        g = table[sub.reshape(-1)].reshape(P, self.VT, width)    # [P, VT, w]
        out = np.empty((P, self.VT * width), table.dtype)
        for (q0, q1, D) in self.runs:
            a, b = self.coff[q0], self.coff[q0] + (q1 - q0) * D
            blk = g[:, a:b, :].reshape(P, q1 - q0, D, width)
            out[:, a * width:b * width] = (
                blk.transpose(0, 1, 3, 2).reshape(P, -1))
        return out


# ---------------------------------------------------------------- kernels
def build_k1(cfg: Cfg):
    """sup.T = (x @ W1).T via psum[64, cols] accumulation.

    f32 precision at bf16 PE rate: x and W1 are split hi/lo in bf16 and
    three of the four cross terms are accumulated (lo*lo ~ 2^-16, dropped).
    """
    H, kb, NP = cfg.hidden, cfg.kb, cfg.NP
    CC = cfg.k1_cols            # DMA chunk columns
    PC = min(512, CC)           # psum sub-chunk columns
    nc = bacc.Bacc(None, target_bir_lowering=False)
    x_d = nc.dram_tensor("xhl", [P, kb, 2, NP], BF16, kind="ExternalInput")
    w1_d = nc.dram_tensor("w1hl", [P, kb, 2, H], BF16, kind="ExternalInput")
    sup_d = nc.dram_tensor("sup", [H, NP], F32, kind="ExternalOutput")

    n_ch = -(-NP // CC)
    with tile.TileContext(nc) as tc:
        with (
            tc.tile_pool(name="const", bufs=1) as cpool,
            tc.tile_pool(name="xload", bufs=6) as xpool,
            tc.tile_pool(name="sout", bufs=1) as opool,
            tc.tile_pool(name="ps", bufs=5, space="PSUM") as pspool,
            tc.tile_pool(name="psw", bufs=1, space="PSUM") as pswarm,
        ):
            w1_t = cpool.tile([P, kb, 2, H], BF16)
            nc.sync.dma_start(w1_t[:], w1_d[:])
            osb = opool.tile([H, NP], F32)
            # ~4.5us of dummy matmuls while the first x chunk is in flight:
            # sustained PE activity flips the HAM clock gate 1.2 -> 2.4 GHz
            # before the real matmuls start (stays warm; PE runs near
            # continuously afterwards).
            ps_w = pswarm.tile([H, H], F32, tag="warm")
            for _ in range(80):
                nc.tensor.matmul(ps_w[:], w1_t[:, 0, 0, :], w1_t[:, 0, 0, :],
                                 start=True, stop=True)
            for i in range(n_ch):
                c0 = i * CC
                ncols = min(CC, NP - c0)
                xc = xpool.tile([P, kb, 2, CC], BF16, tag="xc")
                nc.sync.dma_start(xc[:, :, :, :ncols],
                                  x_d[:, :, :, c0:c0 + ncols])
                for s0 in range(0, ncols, PC):
                    sc = min(PC, ncols - s0)
                    ps = pspool.tile([H, PC], F32, tag="ps")
                    nmm = 3 * kb
                    m = 0
                    for k in range(kb):
                        for hl in (0, 1):           # whi @ {xhi, xlo}
                            nc.tensor.matmul(ps[:, :sc], w1_t[:, k, 0, :],
                                             xc[:, k, hl, s0:s0 + sc],
                                             start=(m == 0), stop=(m == nmm - 1))
                            m += 1
                    for k in range(kb):             # wlo @ xhi
                        nc.tensor.matmul(ps[:, :sc], w1_t[:, k, 1, :],
                                         xc[:, k, 0, s0:s0 + sc],
                                         start=False, stop=(m == nmm - 1))
                        m += 1
                    nc.scalar.activation(osb[:, c0 + s0:c0 + s0 + sc],
                                         ps[:, :sc], CPY)
            nc.sync.dma_start(sup_d[:], osb[:])
    nc.compile()
    return nc


def build_spmm(cfg: Cfg, sch: Sched, layer: int, q_scale: float = 1.0):
    """Slab-streaming spmm. layer=1: +b1, relu, @W2 -> hW2 shard.
    layer=2: +b2, softmax -> out shard."""
    H, C, Q = cfg.hidden, cfg.n_class, cfg.Q
    W = H if layer == 1 else C          # table width
    SLT = F32
    nc = bacc.Bacc(None, target_bir_lowering=False)
    slt_d = nc.dram_tensor("slots", [P, max(sch.VT * W, 1)], SLT,
                           kind="ExternalInput")
    val_d = nc.dram_tensor("valv", [P, max(sch.VT, 1)], F32,
                           kind="ExternalInput")
    if layer == 1:
        b_d = nc.dram_tensor("b1r", [P, H], F32, kind="ExternalInput")
        id_d = nc.dram_tensor("ident", [P, P], F32, kind="ExternalInput")
        w2_d = nc.dram_tensor("w2", [P, 2, C], F32, kind="ExternalInput")
        out_d = nc.dram_tensor("hw2", [P, Q * C], F32, kind="ExternalOutput")
    else:
        b_d = nc.dram_tensor("b2r", [P, C], F32, kind="ExternalInput")
        out_d = nc.dram_tensor("oout", [P, Q * C], F32, kind="ExternalOutput")

    chunks = sch.chunks(W, cfg.chunk_elems)
    nqc_max = max(ch["nqc"] for ch in chunks)
    L_max = max(ch["L"] for ch in chunks)
    seg_max = max(nq * W * D for ch in chunks for (_, nq, D, _) in ch["segs"])

    # greedy balance of the val-multiplies between GpSimd (~1.92 ns/elem,
    # ~2.5us drain overhead per op) and DVE (~1.04 ns/elem + ~0.3us/op,
    # which also owns every reduction)
    GP_NS, DVE_NS, RED_NS, GP_OP, DVE_OP = 1.55, 1.04, 0.72, 1000.0, 300.0
    gp_busy = 0.0
    dve_busy = 15000.0 if layer == 2 else 2000.0    # epilogue handicap
    mult_on_gp = []
    for ch in chunks:
        for (qseg, nq, D, loc) in ch["segs"]:
            E = nq * W * D
            dve_busy += E * RED_NS + DVE_OP         # the reduce
            gp_c = E * GP_NS + GP_OP
            dve_c = E * DVE_NS + DVE_OP
            if gp_busy + gp_c <= dve_busy + dve_c:
                mult_on_gp.append(True)
                gp_busy += gp_c
            else:
                mult_on_gp.append(False)
                dve_busy += dve_c
    with tile.TileContext(nc) as tc:
        with (
            tc.tile_pool(name="const", bufs=1) as cpool,
            tc.tile_pool(name="sld", bufs=10) as spool,
            tc.tile_pool(name="acc", bufs=3) as apool,
            tc.tile_pool(name="epi", bufs=3) as epool,
            tc.tile_pool(name="ob", bufs=1) as opool,
            tc.tile_pool(name="psA", bufs=4, space="PSUM") as psA,
            tc.tile_pool(name="psB", bufs=4, space="PSUM") as psB,
        ):
            val_t = cpool.tile([P, max(sch.VT, 1)], F32)
            nc.sync.dma_start(val_t[:], val_d[:])
            b_t = cpool.tile([P, H if layer == 1 else C], F32)
            nc.sync.dma_start(b_t[:], b_d[:])
            if layer == 1:
                id_t = cpool.tile([P, P], F32)
                w2_t = cpool.tile([P, 2, C], F32)
                nc.sync.dma_start(id_t[:], id_d[:])
                nc.sync.dma_start(w2_t[:], w2_d[:])
            ob = opool.tile([P, Q, C], F32)
            if layer == 2:
                lg = opool.tile([P, Q, C], F32)
            else:
                hb = opool.tile([P, Q, H], F32)

            def finish_chunk(ch, acc_c):
                """Per-chunk epilogue once all its reduces are emitted."""
                q0c, nqc = ch["q0c"], ch["nqc"]
                # +b1 into the global h tile (frees acc_c immediately; PE/ACT
                # lag can't back-pressure the reduce pipeline), relu, @W2.
                hv = hb[:, q0c:q0c + nqc, :]
                nc.vector.tensor_tensor(
                    hv, acc_c[:, :nqc, :],
                    b_t[:].unsqueeze(1).broadcast_to([P, nqc, W]), op=ADD)
                nc.scalar.activation(
                    hv.rearrange("p q w -> p (q w)"),
                    hv.rearrange("p q w -> p (q w)"), RELU)
                for jj in range(0, nqc, 2):
                    nj = min(2, nqc - jj)
                    # one transpose covers two h columns (F=128)
                    ps2 = psA.tile([P, P], F32, tag="tr")
                    nc.tensor.transpose(
                        ps2[:nj * H, :],
                        hb[:, q0c + jj:q0c + jj + nj, :].rearrange(
                            "p a b -> p (a b)"), id_t[:])
                    hT = epool.tile([P, P], F32, tag="hT")
                    nc.scalar.activation(hT[:nj * H, :], ps2[:nj * H, :], CPY)
                    ps3 = psB.tile([P, 2, C], F32, tag="mm")
                    for j in range(nj):
                        nc.tensor.matmul(ps3[:, j, :], hT[:, :],
                                         w2_t[:, j, :], start=True, stop=True)
                    nc.scalar.activation(ob[:, q0c + jj:q0c + jj + nj, :],
                                         ps3[:, :nj, :], CPY)

            # software pipeline at segment granularity: each segment gets
            # its own DMA + multiply; its reduce is emitted LAG segments
            # later so neither engine head-blocks on a lagging producer.
            LAG = 6 if layer == 1 else 3
            from collections import deque
            segq = deque()
            grp = {}

            def drain_one():
                ci, qseg, nq, D, sv = segq.popleft()
                ch = chunks[ci]
                if layer == 1:
                    acc_c, left = grp[ci]
                    dst = acc_c[:, qseg - ch["q0c"]:qseg - ch["q0c"] + nq, :]
                else:
                    dst = lg[:, qseg:qseg + nq, :]
                nc.vector.tensor_reduce(dst, sv, axis=AX, op=ADD)
                if layer == 1:
                    grp[ci][1] -= 1
                    if grp[ci][1] == 0:
                        finish_chunk(ch, grp.pop(ci)[0])

            seg_i = 0
            for ci, ch in enumerate(chunks):
                if layer == 1:
                    acc_c = apool.tile([P, nqc_max, W], F32, tag="acc")
                    grp[ci] = [acc_c, len(ch["segs"])]
                for (qseg, nq, D, loc) in ch["segs"]:
                    L = nq * W * D
                    sl = spool.tile([P, seg_max], SLT, tag="sl")
                    e0 = ch["eoff"] + loc
                    nc.sync.dma_start(sl[:, :L], slt_d[:, e0:e0 + L])
                    # drain a lagged reduce BEFORE this segment's multiply:
                    # its producer finished LAG segs ago, so the in-order DVE
                    # head never blocks on this segment's DMA while ready
                    # reduce work exists.
                    if len(segq) > LAG:
                        drain_one()
                    v4 = sl[:, :L].rearrange(
                        "p (q h d) -> p q h d", q=nq, h=W, d=D)
                    vw = (val_t[:, e0 // W:e0 // W + nq * D]
                          .rearrange("p (q d) -> p q d", q=nq)
                          .unsqueeze(2).broadcast_to([P, nq, W, D]))
                    o4 = v4
                    eng = nc.gpsimd if mult_on_gp[seg_i] else nc.vector
                    seg_i += 1
                    eng.tensor_tensor(o4, v4, vw, op=MUL)
                    segq.append((ci, qseg, nq, D, o4))
            while segq:
                drain_one()

            if layer == 2:
                flat = lg[:].rearrange("p q w -> p (q w)")
                nc.vector.tensor_tensor(
                    lg[:], lg[:],
                    b_t[:].unsqueeze(1).broadcast_to([P, Q, C]), op=ADD)
                nm = epool.tile([P, Q], F32, tag="nm")
                nc.vector.reduce_max(nm[:], lg[:], axis=AX, negate=True)
                nc.vector.tensor_tensor(
                    lg[:], lg[:],
                    nm[:].unsqueeze(2).broadcast_to([P, Q, C]), op=ADD)
                nc.scalar.activation(flat, flat, EXP)
                se = epool.tile([P, Q], F32, tag="se")
                nc.vector.reduce_sum(se[:], lg[:], axis=AX)
                ri = epool.tile([P, Q], F32, tag="ri")
                nc.vector.reciprocal(ri[:], se[:])
                nc.vector.tensor_tensor(
                    ob[:], lg[:],
                    ri[:].unsqueeze(2).broadcast_to([P, Q, C]), op=MUL)
            nc.sync.dma_start(out_d[:], ob[:].rearrange("p q c -> p (q c)"))
    nc.compile()
    return nc


# ---------------------------------------------------------------- driver
LAST_PROFILE = {}


def _run(nc, in_maps, label):
    trace = os.environ.get("GCN_PROFILE") == "1"
    t0 = time.time()
    res = bass_utils.run_bass_kernel_spmd(
        nc, in_maps, core_ids=list(range(len(in_maps))), trace=trace)
    LAST_PROFILE[label] = dict(wall_s=time.time() - t0,
                               exec_time_ns=res.exec_time_ns,
                               trace=(res.instructions_and_trace or (None, None))[1])
    return res.results


def gcn_forward(cfg: Cfg, x, edge_src, edge_dst, edge_val, W1, b1, W2, b2):
    ncr, H, C, Q, npc = cfg.n_cores, cfg.hidden, cfg.n_class, cfg.Q, cfg.npc
    x = np.asarray(x, np.float32)
    W1 = np.asarray(W1, np.float32)
    b1 = np.asarray(b1, np.float32)
    W2 = np.asarray(W2, np.float32)
    b2 = np.asarray(b2, np.float32)
    edge_src = np.asarray(edge_src, np.int64)
    edge_dst = np.asarray(edge_dst, np.int64)
    edge_val = np.asarray(edge_val, np.float32)

    t0 = time.time()
    sch = Sched(cfg, edge_src, edge_dst, edge_val)
    prep_s = time.time() - t0

    import ml_dtypes
    BF = ml_dtypes.bfloat16
    ident = np.eye(P, dtype=np.float32)
    b1r = np.tile(b1, (P, 1))
    b2r = np.tile(b2, (P, 1))
    w1r = np.ascontiguousarray(W1.reshape(cfg.kb, P, H).transpose(1, 0, 2))
    w2sel = np.zeros((P, 2, C), np.float32)
    w2sel[:H, 0] = W2
    w2sel[H:2 * H, 1] = W2
    w1hi = w1r.astype(BF)
    w1lo = (w1r - w1hi.astype(np.float32)).astype(BF)
    w1hl = np.ascontiguousarray(np.stack([w1hi, w1lo], axis=2))

    # K1: sup = x @ W1 (transposed output [H, NP] per core)
    in1 = []
    for c in range(ncr):
        xs = x[c * npc:(c + 1) * npc]
        xt = np.zeros((P, cfg.kb, cfg.NP), np.float32)
        xt[:, :, :npc] = xs.T.reshape(cfg.kb, P, npc).transpose(1, 0, 2)
        xhi = xt.astype(BF)
        xlo = (xt - xhi.astype(np.float32)).astype(BF)
        in1.append(dict(xhl=np.ascontiguousarray(np.stack([xhi, xlo], axis=2)),
                        w1hl=w1hl))
    nc1 = build_k1(cfg)
    r1 = _run(nc1, in1, "k1")

    sup = np.empty((cfg.n_nodes, H), np.float32)
    for c in range(ncr):
        sup[c * npc:(c + 1) * npc] = r1[c]["sup"].T[:npc]

    # K2: slab spmm + bias + relu + @W2
    in2 = [dict(slots=sch.build_slab(c, sup, H), valv=sch.valmat[c],
                b1r=b1r, ident=ident, w2=w2sel)
           for c in range(ncr)]
    nc2 = build_spmm(cfg, sch, 1)
    r2 = _run(nc2, in2, "k2")

    hw2 = np.empty((cfg.n_nodes, C), np.float32)
    for c in range(ncr):
        flat = r2[c]["hw2"].reshape(P, Q, C).transpose(1, 0, 2).reshape(-1, C)
        o = sch.order[c]
        m = o < npc
        hw2[c * npc + o[m]] = flat[m]

    # K3: slab spmm + bias + softmax
    in3 = [dict(slots=sch.build_slab(c, hw2, C), valv=sch.valmat[c], b2r=b2r)
           for c in range(ncr)]
    nc3 = build_spmm(cfg, sch, 2)
    r3 = _run(nc3, in3, "k3")

    out = np.empty((cfg.n_nodes, C), np.float32)
    for c in range(ncr):
        flat = r3[c]["oout"].reshape(P, Q, C).transpose(1, 0, 2).reshape(-1, C)
        o = sch.order[c]
        m = o < npc
        out[c * npc + o[m]] = flat[m]

    LAST_PROFILE["prep_s"] = prep_s
    LAST_PROFILE["sched"] = dict(VT=sch.VT, runs=len(sch.runs),
                                 n_chunks2=len(sch.chunks(H, cfg.chunk_elems)),
                                 pad=float(sch.VT * P * ncr) / max(len(edge_src), 1))
    return out


def kernel(x, edge_src, edge_dst, edge_val, W1, b1, W2, b2):
    cfg = Cfg()
    return gcn_forward(cfg, x, edge_src, edge_dst, edge_val, W1, b1, W2, b2)


# ---------------------------------------------------------------- self test
def _numpy_ref(x, es, ed, ev, W1, b1, W2, b2, n):
    def spmm(d):
        g = d[es] * ev[:, None]
        out = np.zeros((n, d.shape[1]), np.float32)
        np.add.at(out, ed, g)
        return out
    h = spmm(x @ W1) + b1
    h = np.maximum(h, 0)
    lg = spmm(h @ W2) + b2
    e = np.exp(lg - lg.max(1, keepdims=True))
    return e / e.sum(1, keepdims=True)


def _selftest():
    cfg = Cfg(n_nodes=4096, f_in=256, hidden=64, n_class=16, n_cores=8,
              chunk_elems=2048, k1_cols=256)
    rng = np.random.default_rng(1)
    n_edges = 65536
    x = rng.standard_normal((cfg.n_nodes, cfg.f_in), dtype=np.float32)
    es = rng.integers(0, cfg.n_nodes, n_edges)
    ed = rng.integers(0, cfg.n_nodes, n_edges)
    ev = rng.random(n_edges, dtype=np.float32)
    W1 = rng.standard_normal((cfg.f_in, cfg.hidden), dtype=np.float32) * 0.125
    b1 = rng.standard_normal(cfg.hidden, dtype=np.float32) * 0.01
    W2 = rng.standard_normal((cfg.hidden, cfg.n_class), dtype=np.float32) * 0.25
    b2 = rng.standard_normal(cfg.n_class, dtype=np.float32) * 0.01
    act = gcn_forward(cfg, x, es, ed, ev, W1, b1, W2, b2)
    ref = _numpy_ref(x, es, ed, ev, W1, b1, W2, b2, cfg.n_nodes)
    err = np.abs(act - ref).max()
    rel = err / np.abs(ref).max()
    print(f"selftest absmax={err:.3e} relmax={rel:.3e}")
    print("profile:", LAST_PROFILE)
    assert rel < 1e-3, "SELFTEST FAIL"
    print("SELFTEST PASS")


if __name__ == "__main__":
    _selftest()
